# revision 17
# baseline (speedup 1.0000x reference)
"""Trainium2 Bass kernel for 2-layer RGCN (nn_PygModel_52003464020165).

Self-contained: accepts FULL inputs, shards across 8 NeuronCores internally,
returns FULL [64, 10] output.

Architecture (per core, dst-sharded graph):
  - ALL per-core inputs packed into ONE u8 blob (host->device transfer over
    the axon tunnel is the wall-clock bottleneck: ~35 MB/s + ~90ms/array).
    x ships as fp8(e4m3), edge slots as 5 bytes (u16+u8 src idx, u8 loc,
    u8 count -> alpha via device reciprocal), relation weights sharded
    across cores and AllGathered, iota/identity generated on device.
  - full h replicated each layer via AllGather (bf16, [N, H] row-major DRAM)
  - per dst-chunk (512 dense dst cols): batched indirect-DMA gather of
    h[src] rows -> msg tiles [128 edges, H] (edges on partitions)
  - per relation r: alpha-hot matrices [128 edges, 128 win] built by one DVE
    tensor_scalar (is_equal vs iota, scaled by 1/cnt); PE matmuls
    msg^T @ alphahot accumulate mean bins into PSUM [H, chunk]
  - transform: root matmul + 20 relation matmuls (W_r stationary, bf16)
    accumulate out^T [H, chunk] in PSUM; evacuation fuses BN partial stats
  - BatchNorm stats via AllReduce; affine+ReLU as one ACT op over [H, NS]
  - PE transposes h^T -> row-major shard -> DRAM -> AllGather
  - global mean pool via indicator matmuls + AllReduce; final linear+sigmoid
"""

import math
import sys

sys.path.insert(0, "/opt/trn_rl_repo")

import ml_dtypes
import numpy as np

# Persistent XLA compilation cache: run_bass_kernel_spmd re-jits a fresh
# closure on every call, so without this each call pays ~1.2s of XLA
# compile; with it the recompile is a ~30ms disk-cache hit.
try:
    import jax as _jax

    _jax.config.update("jax_compilation_cache_dir", "/tmp/jax_comp_cache")
    _jax.config.update("jax_persistent_cache_min_compile_time_secs", 0.0)
    _jax.config.update("jax_persistent_cache_min_entry_size_bytes", 0)
except Exception:
    pass

import concourse.bacc as bacc
import concourse.bass as bass
import concourse.tile as tile
from concourse import mybir
from concourse.bass_utils import run_bass_kernel_spmd

BF16 = ml_dtypes.bfloat16
FP8 = ml_dtypes.float8_e4m3
P = 128
ALIGN = 512


class Cfg:
    def __init__(self, N=100000, E=1600000, F=64, H=128, R=20, G=64, C=10, L=2,
                 NC=8, CHUNK=512, WIN=128, EPS=1e-5, DT="bf16", DEBUG=False):
        assert H == P
        self.N, self.E, self.F, self.H, self.R, self.G, self.C, self.L = (
            N, E, F, H, R, G, C, L)
        self.NC, self.CHUNK, self.WIN, self.EPS = NC, CHUNK, WIN, EPS
        self.DT = DT
        self.DEBUG = DEBUG
        assert N % NC == 0
        self.NS = N // NC
        self.nchunks = math.ceil(self.NS / CHUNK)
        self.cw = [min(CHUNK, self.NS - c * CHUNK) for c in range(self.nchunks)]
        self.nwin = [math.ceil(w / WIN) for w in self.cw]
        self.nblk = math.ceil(self.NS / P)
        # relation-weight stack: L*R rel mats + L root mats, padded so each
        # core ships an equal partition-slice
        self.NMAT = L * R + L
        self.MATCOLS = self.NMAT * H              # 42*128 = 5376
        self.PSL = P // NC                        # partition rows per core


def _aligned_layout(fields):
    """fields: list of (name, nbytes). Returns (offsets dict, total)."""
    off = {}
    cur = 0
    for name, nb in fields:
        cur = (cur + ALIGN - 1) // ALIGN * ALIGN
        off[name] = cur
        cur += nb
    total = (cur + ALIGN - 1) // ALIGN * ALIGN
    return off, total


def _layout(cfg, S_total):
    F, H, R, G, C, L, NS = cfg.F, cfg.H, cfg.R, cfg.G, cfg.C, cfg.L, cfg.NS
    fields = [
        ("x8", F * NS),                 # fp8 [F, NS]
        ("lo", P * S_total * 2),        # u16 [P, S] low 16 idx bits
        ("loc", P * S_total),           # u8 [P, S]: loc | (idx_hi << 7)
        ("cnt", P * S_total),           # u8 [P, S]: in-degree, 0 = sentinel
        ("relw", cfg.PSL * cfg.MATCOLS * 2),  # bf16 [PSL, MATCOLS]
        ("w_in", F * H * 2),            # bf16 [F, H]
        ("gids", P * cfg.nblk),         # u8 [P, nblk]
        ("invg", C * G * 4),            # f32 [C, G]
        ("bng", H * L * 4),             # f32 [H, L]
        ("bnb", H * L * 4),             # f32 [H, L]
        ("b_in", H * 4),                # f32 [H, 1]
        ("w_out", H * C * 4),           # f32 [H, C]
        ("b_out", C * 4),               # f32 [C, 1]
    ]
    return _aligned_layout(fields)


def _plan(cfg, edge_index, edge_type, batch):
    """Host-side planner. Returns shared structure + per-core data arrays."""
    N, R, NC, NS, CHUNK, WIN = cfg.N, cfg.R, cfg.NC, cfg.NS, cfg.CHUNK, cfg.WIN
    src = edge_index[0].astype(np.int64)
    dst = edge_index[1].astype(np.int64)
    et = edge_type.astype(np.int64)

    comb = dst * R + et
    cnt = np.bincount(comb, minlength=N * R)
    cnt_e = np.maximum(cnt[comb], 1)
    assert cnt_e.max() < 256

    core = dst // NS
    dloc = dst % NS
    chunk = dloc // CHUNK
    inchunk = dloc % CHUNK
    win = inchunk // WIN
    loc = (inchunk % WIN).astype(np.int64)

    maxwin = max(cfg.nwin)
    gid = (chunk * R + et) * maxwin + win
    ngroups = cfg.nchunks * R * maxwin

    counts = np.zeros((NC, ngroups), np.int64)
    np.add.at(counts, (core, gid), 1)
    Tg = np.maximum(1, -(-counts.max(axis=0) // P))  # ceil div, min 1

    # tile order: chunk-major, then r, then win
    group_order = []
    for c in range(cfg.nchunks):
        for r in range(R):
            for w in range(cfg.nwin[c]):
                group_order.append((c * R + r) * maxwin + w)
    group_order = np.array(group_order, np.int64)
    tiles_of_group = Tg[group_order]
    tile_base = np.zeros(len(group_order), np.int64)
    np.cumsum(tiles_of_group[:-1], out=tile_base[1:])
    S_total = int(tiles_of_group.sum())

    gpos = np.full(ngroups, -1, np.int64)
    gpos[group_order] = np.arange(len(group_order))

    locA = np.zeros((NC, P, S_total), np.uint8)  # loc | (idx_hi << 7)
    cntA = np.zeros((NC, P, S_total), np.uint8)  # 0 = sentinel (alpha -> 0)
    srcA = np.zeros((NC, P, S_total), np.int64)  # sentinel: gather row 0

    order = np.lexsort((gid, core))
    s_core, s_gid = core[order], gid[order]
    s_src, s_loc, s_cnt = src[order], loc[order], cnt_e[order]
    key = s_core * ngroups + s_gid
    first = np.r_[True, key[1:] != key[:-1]]
    grp_start = np.flatnonzero(first)
    seglen = np.diff(np.r_[grp_start, len(key)])
    rank = np.arange(len(key)) - np.repeat(grp_start, seglen)

    slot = tile_base[gpos[s_gid]] * P + rank
    srcA[s_core, slot % P, slot // P] = s_src
    locA[s_core, slot % P, slot // P] = s_loc + 128 * (s_src >> 16)
    cntA[s_core, slot % P, slot // P] = s_cnt

    # emission structure: per chunk -> (slot_lo, slot_hi,
    #   per-r list of per-win (tile_base, ntiles))
    chunk_tiles = []
    for c in range(cfg.nchunks):
        lo = None
        hi = 0
        rlists = []
        for r in range(R):
            wl = []
            for w in range(cfg.nwin[c]):
                pos = gpos[(c * R + r) * maxwin + w]
                tb, tn = int(tile_base[pos]), int(tiles_of_group[pos])
                if lo is None:
                    lo = tb
                hi = tb + tn
                wl.append((tb, tn, w))
            rlists.append(wl)
        chunk_tiles.append((lo, hi, rlists))

    loA = (srcA & 0xFFFF).astype(np.uint16)
    assert srcA.max() < (1 << 17)

    gcnt = np.bincount(batch.astype(np.int64), minlength=cfg.G).astype(np.float32)
    inv_gcnt = 1.0 / np.maximum(gcnt, 1.0)

    return dict(S_total=S_total, chunk_tiles=chunk_tiles, locA=locA,
                cntA=cntA, loA=loA, inv_gcnt=inv_gcnt)


def _build_nc(cfg, plan):
    """Emit the SPMD Bass program (one program, NC cores)."""
    N, F, H, R, G, C, L = cfg.N, cfg.F, cfg.H, cfg.R, cfg.G, cfg.C, cfg.L
    NS, CHUNK, WIN = cfg.NS, cfg.CHUNK, cfg.WIN
    S_total = plan["S_total"]
    chunk_tiles = plan["chunk_tiles"]
    nblk = cfg.nblk
    OFF, TOT = _layout(cfg, S_total)

    nc = bacc.Bacc(None)
    f32, i32 = mybir.dt.float32, mybir.dt.int32
    u8, u16 = mybir.dt.uint8, mybir.dt.uint16
    bf16 = mybir.dt.bfloat16
    f8 = mybir.dt.float8e4
    AF = mybir.ActivationFunctionType
    OP = mybir.AluOpType
    assert cfg.DT == "bf16"

    blob_d = nc.dram_tensor("blob", [TOT], u8, kind="ExternalInput")
    out_d = nc.dram_tensor("out", [C, G], f32, kind="ExternalOutput")

    def v(name, shape, dt_):
        nbytes = math.prod(shape) * mybir.dt.size(dt_)
        ap = blob_d[OFF[name]:OFF[name] + nbytes].bitcast(dt_)
        if len(shape) == 2:
            ap = ap.rearrange("(a b) -> a b", b=shape[1])
        return ap

    relw_in = nc.dram_tensor("relw_in", [cfg.PSL, cfg.MATCOLS], bf16)
    relw_full = nc.dram_tensor("relw_full", [P, cfg.MATCOLS], bf16,
                               addr_space="Shared")
    h_shard = [nc.dram_tensor(f"h_shard{l}", [NS, H], bf16) for l in range(L)]
    h_full = [nc.dram_tensor(f"h_full{l}", [N, H], bf16, addr_space="Shared")
              for l in range(L)]
    stats_in = nc.dram_tensor("stats_in", [H, 2], f32)
    stats_out = nc.dram_tensor("stats_out", [H, 2], f32, addr_space="Shared")
    pool_in = nc.dram_tensor("pool_in", [G, H], f32)
    pool_out = nc.dram_tensor("pool_out", [G, H], f32, addr_space="Shared")
    if cfg.DEBUG:
        dbg_h = [nc.dram_tensor(f"dbg_h{l}", [N, H], bf16,
                                kind="ExternalOutput") for l in range(L)]
        dbg_outb = nc.dram_tensor("dbg_outb", [L, H, NS], bf16,
                                  kind="ExternalOutput")
        dbg_stg = nc.dram_tensor("dbg_stg", [L, H, 8], f32,
                                 kind="ExternalOutput")
        dbg_pool = nc.dram_tensor("dbg_pool", [G, H], f32,
                                  kind="ExternalOutput")

    cores = list(range(cfg.NC))

    with tile.TileContext(nc) as tc:
        with (
            tc.tile_pool(name="const", bufs=1) as cpool,
            tc.tile_pool(name="big", bufs=1) as bigpool,
            tc.tile_pool(name="msg", bufs=2) as msgpool,
            tc.tile_pool(name="hot", bufs=16) as hotpool,
            tc.tile_pool(name="mean", bufs=2) as meanpool,
            tc.tile_pool(name="work", bufs=3) as workpool,
            tc.tile_pool(name="psA", bufs=2, space="PSUM") as psA,
            tc.tile_pool(name="psT", bufs=2, space="PSUM") as psT,
            tc.tile_pool(name="psB", bufs=2, space="PSUM") as psB,
        ):
            # ---------- relation-weight AllGather (tiny; kick off first) ----
            nc.sync.dma_start(relw_in[:], v("relw", [cfg.PSL, cfg.MATCOLS],
                                            bf16))
            nc.gpsimd.collective_compute(
                "AllGather", OP.bypass, replica_groups=[cores],
                ins=[relw_in[:]], outs=[relw_full[:]])
            relw_t = cpool.tile([P, cfg.NMAT, H], bf16, tag="relw")
            nc.sync.dma_start(
                relw_t[:], relw_full[:].rearrange("p (m h) -> p m h", h=H))

            # ---------- generated constants ----------
            iota_bf = cpool.tile([P, WIN], bf16, tag="iota_bf")
            nc.gpsimd.iota(iota_bf[:], pattern=[[1, WIN]], base=0,
                           channel_multiplier=0,
                           allow_small_or_imprecise_dtypes=True)
            idiag = cpool.tile([P, P], i32, tag="idiag")
            nc.gpsimd.iota(idiag[:], pattern=[[1, P]], base=0,
                           channel_multiplier=-1)
            ident = cpool.tile([P, P], bf16, tag="ident")
            nc.vector.tensor_scalar(out=ident[:], in0=idiag[:], scalar1=0,
                                    scalar2=None, op0=OP.is_equal)
            identf = cpool.tile([G, G], f32, tag="identf")
            nc.vector.tensor_scalar(out=identf[:], in0=idiag[:G, :G],
                                    scalar1=0, scalar2=None, op0=OP.is_equal)

            # ---------- edge-data unpack ----------
            # planes: lo = low 16 idx bits; lochi = loc | (idx_hi << 7);
            # cnt = in-degree with 0 as the padding sentinel
            lo_t = cpool.tile([P, S_total], u16, tag="lo")
            nc.sync.dma_start(lo_t[:], v("lo", [P, S_total], u16))
            loc8_t = cpool.tile([P, S_total], u8, tag="loc8")
            nc.sync.dma_start(loc8_t[:], v("loc", [P, S_total], u8))
            cnt8_t = cpool.tile([P, S_total], u8, tag="cnt8")
            nc.sync.dma_start(cnt8_t[:], v("cnt", [P, S_total], u8))

            loc_t = cpool.tile([P, S_total], f32, tag="loc")
            alp_t = cpool.tile([P, S_total], f32, tag="alp")
            scr_t = cpool.tile([P, S_total], f32, tag="scr")
            idx_t = cpool.tile([P, S_total], i32, tag="idx")
            # (all arithmetic exact in f32: values < 2^17)
            nc.vector.tensor_copy(scr_t[:], loc8_t[:])          # byte
            nc.vector.tensor_scalar(out=alp_t[:], in0=scr_t[:],
                                    scalar1=128.0, scalar2=None,
                                    op0=OP.is_ge)               # idx_hi
            nc.vector.tensor_copy(loc_t[:], lo_t[:])            # idx_lo
            nc.vector.tensor_scalar(out=alp_t[:], in0=alp_t[:],
                                    scalar1=65536.0, scalar2=None,
                                    op0=OP.mult)                # idx_hi<<16
            nc.vector.tensor_tensor(out=loc_t[:], in0=loc_t[:], in1=alp_t[:],
                                    op=OP.add)                  # idx
            nc.vector.tensor_copy(idx_t[:], loc_t[:])
            nc.vector.tensor_scalar(out=alp_t[:], in0=alp_t[:],
                                    scalar1=1.0 / 512.0, scalar2=None,
                                    op0=OP.mult)                # idx_hi<<7
            nc.vector.tensor_tensor(out=loc_t[:], in0=scr_t[:], in1=alp_t[:],
                                    op=OP.subtract)             # loc
            nc.vector.tensor_copy(alp_t[:], cnt8_t[:])          # cnt
            nc.vector.tensor_scalar(out=scr_t[:], in0=alp_t[:],
                                    scalar1=0.0, scalar2=None,
                                    op0=OP.is_gt)               # pad mask
            nc.vector.tensor_scalar(out=alp_t[:], in0=alp_t[:],
                                    scalar1=1.0, scalar2=None,
                                    op0=OP.max)
            nc.vector.reciprocal(alp_t[:], alp_t[:])
            nc.vector.tensor_tensor(out=alp_t[:], in0=alp_t[:], in1=scr_t[:],
                                    op=OP.mult)                 # alpha

            # ---------- small params ----------
            w_in_t = cpool.tile([F, H], bf16, tag="w_in")
            nc.sync.dma_start(w_in_t[:], v("w_in", [F, H], bf16))
            b_in_t = cpool.tile([H, 1], f32, tag="b_in")
            nc.sync.dma_start(b_in_t[:], v("b_in", [H, 1], f32))
            bng_t = cpool.tile([H, L], f32, tag="bng")
            nc.sync.dma_start(bng_t[:], v("bng", [H, L], f32))
            bnb_t = cpool.tile([H, L], f32, tag="bnb")
            nc.sync.dma_start(bnb_t[:], v("bnb", [H, L], f32))
            gids8_t = cpool.tile([P, nblk], u8, tag="gids8")
            nc.sync.dma_start(gids8_t[:], v("gids", [P, nblk], u8))
            gids_t = cpool.tile([P, nblk], f32, tag="gids")
            nc.vector.tensor_copy(gids_t[:], gids8_t[:])
            w_out_t = cpool.tile([H, C], f32, tag="w_out")
            nc.sync.dma_start(w_out_t[:], v("w_out", [H, C], f32))
            b_out_t = cpool.tile([C, 1], f32, tag="b_out")
            nc.sync.dma_start(b_out_t[:], v("b_out", [C, 1], f32))
            invg_t = cpool.tile([C, G], f32, tag="invg")
            nc.sync.dma_start(invg_t[:], v("invg", [C, G], f32))

            # fences: pull const-load DMA completions into engine program
            # order one DMA at a time, so compute ops (tiny ISA wait
            # budgets) emit no DMA waits of their own
            fence = cpool.tile([1, 1], f32, tag="fence")
            for _ft in (bng_t, bnb_t, invg_t, w_out_t):
                nc.vector.tensor_copy(fence[:], _ft[0:1, 0:1])
            fenceA = cpool.tile([1, 1], f32, tag="fenceA")
            for _ft in (b_in_t, b_out_t):
                nc.scalar.copy(fenceA[:], _ft[0:1, 0:1])
            nc.scalar.copy(fenceA[:], relw_t[0:1, 0, 0:2].bitcast(f32))
            nc.scalar.copy(fenceA[:], w_in_t[0:1, 0:2].bitcast(f32))

            hT = bigpool.tile([P, NS], bf16, tag="hT")
            outb = bigpool.tile([P, NS], bf16, tag="outb")
            sum_parts = bigpool.tile([P, cfg.nchunks], f32, tag="sumP")
            sq_parts = bigpool.tile([P, cfg.nchunks], f32, tag="sqP")
            sq_scr = bigpool.tile([P, CHUNK], bf16, tag="sqscr")

            # ---------- input MLP ----------
            xv = v("x8", [F, NS], f8)
            for c in range(cfg.nchunks):
                cw = cfg.cw[c]
                xc8 = workpool.tile([F, CHUNK], f8, tag="xc8")
                nc.sync.dma_start(xc8[:, :cw], xv[:, c * CHUNK:c * CHUNK + cw])
                xc = workpool.tile([F, CHUNK], bf16, tag="xc")
                nc.vector.tensor_copy(xc[:, :cw], xc8[:, :cw])
                ps = psB.tile([P, CHUNK], f32, tag="psB")
                nc.tensor.matmul(out=ps[:, :cw], lhsT=w_in_t[:], rhs=xc[:, :cw],
                                 start=True, stop=True)
                nc.scalar.activation(hT[:, c * CHUNK:c * CHUNK + cw], ps[:, :cw],
                                     AF.Relu, bias=b_in_t[:, 0:1], scale=1.0)

            def emit_transpose_store(l):
                for b in range(nblk):
                    bw = min(P, NS - b * P)
                    pst = psT.tile([P, P], bf16, tag="psT")
                    nc.tensor.transpose(pst[:bw, :P], hT[:, b * P:b * P + bw],
                                        ident[:])
                    rm = workpool.tile([P, P], bf16, tag="rm")
                    nc.vector.tensor_copy(rm[:bw, :], pst[:bw, :P])
                    nc.sync.dma_start(h_shard[l][b * P:b * P + bw, :], rm[:bw, :])
                nc.gpsimd.collective_compute(
                    "AllGather", OP.bypass, replica_groups=[cores],
                    ins=[h_shard[l][:]], outs=[h_full[l][:]])
                if cfg.DEBUG:
                    nc.gpsimd.dma_start(dbg_h[l][:], h_full[l][:])

            emit_transpose_store(0)

            # ---------- RGCN layers ----------
            for l in range(L):
                root_i = L * R + l
                for c in range(cfg.nchunks):
                    cw = cfg.cw[c]
                    lo, hi, rlists = chunk_tiles[c]
                    nS = hi - lo
                    msg = msgpool.tile([P, nS, H], bf16, tag="msg")
                    for s in range(nS):
                        nc.gpsimd.indirect_dma_start(
                            out=msg[:, s, :], out_offset=None,
                            in_=h_full[l][:],
                            in_offset=bass.IndirectOffsetOnAxis(
                                ap=idx_t[:, lo + s:lo + s + 1], axis=0))

                    mean = meanpool.tile([P, R, CHUNK], bf16, tag="mean")
                    for r in range(R):
                        psa = psA.tile([P, CHUNK], f32, tag="psA")
                        for (tb, tn, w) in rlists[r]:
                            ww = min(WIN, cw - w * WIN)
                            for t in range(tn):
                                s = tb + t
                                hot = hotpool.tile([P, WIN], bf16, tag="hot")
                                nc.vector.tensor_scalar(
                                    out=hot[:, :ww], in0=iota_bf[:, :ww],
                                    scalar1=loc_t[:, s:s + 1],
                                    scalar2=alp_t[:, s:s + 1],
                                    op0=OP.is_equal, op1=OP.mult)
                                nc.tensor.matmul(
                                    out=psa[:, w * WIN:w * WIN + ww],
                                    lhsT=msg[:, s - lo, :], rhs=hot[:, :ww],
                                    start=(t == 0), stop=(t == tn - 1))
                        if r % 2 == 0:
                            nc.vector.tensor_copy(mean[:, r, :cw], psa[:, :cw])
                        else:
                            nc.scalar.copy(mean[:, r, :cw], psa[:, :cw])

                    psb = psB.tile([P, CHUNK], f32, tag="psB")
                    nc.tensor.matmul(out=psb[:, :cw], lhsT=relw_t[:, root_i, :],
                                     rhs=hT[:, c * CHUNK:c * CHUNK + cw],
                                     start=True, stop=False)
                    for r in range(R):
                        nc.tensor.matmul(out=psb[:, :cw],
                                         lhsT=relw_t[:, l * R + r, :],
                                         rhs=mean[:, r, :cw],
                                         start=False, stop=(r == R - 1))

                    nc.vector.tensor_scalar(
                        out=outb[:, c * CHUNK:c * CHUNK + cw], in0=psb[:, :cw],
                        scalar1=1.0, scalar2=None, op0=OP.mult, op1=OP.add,
                        accum_out=sum_parts[:, c:c + 1])
                    nc.scalar.activation(sq_scr[:, :cw], psb[:, :cw], AF.Square,
                                         accum_out=sq_parts[:, c:c + 1])

                # ---------- BatchNorm + ReLU ----------
                st = workpool.tile([H, 2], f32, tag="stats")
                nc.vector.reduce_sum(st[:, 0:1], sum_parts[:],
                                     axis=mybir.AxisListType.X)
                nc.vector.reduce_sum(st[:, 1:2], sq_parts[:],
                                     axis=mybir.AxisListType.X)
                nc.sync.dma_start(stats_in[:], st[:])
                nc.gpsimd.collective_compute(
                    "AllReduce", OP.add, replica_groups=[cores],
                    ins=[stats_in[:]], outs=[stats_out[:]])
                stg = workpool.tile([H, 8], f32, tag="stg")
                nc.sync.dma_start(stg[:, 0:2], stats_out[:])
                nc.vector.tensor_scalar(out=stg[:, 2:3], in0=stg[:, 0:1],
                                        scalar1=1.0 / N, scalar2=None,
                                        op0=OP.mult)
                nc.vector.tensor_scalar(out=stg[:, 3:4], in0=stg[:, 1:2],
                                        scalar1=1.0 / N, scalar2=None,
                                        op0=OP.mult)
                nc.vector.tensor_tensor(out=stg[:, 4:5], in0=stg[:, 2:3],
                                        in1=stg[:, 2:3], op=OP.mult)
                nc.vector.tensor_tensor(out=stg[:, 4:5], in0=stg[:, 3:4],
                                        in1=stg[:, 4:5], op=OP.subtract)
                nc.vector.tensor_scalar(out=stg[:, 4:5], in0=stg[:, 4:5],
                                        scalar1=cfg.EPS, scalar2=None,
                                        op0=OP.add)
                nc.scalar.sqrt(stg[:, 5:6], stg[:, 4:5])
                nc.vector.reciprocal(stg[:, 6:7], stg[:, 5:6])
                nc.vector.tensor_tensor(out=stg[:, 6:7], in0=stg[:, 6:7],
                                        in1=bng_t[:, l:l + 1], op=OP.mult)
                nc.vector.tensor_tensor(out=stg[:, 7:8], in0=stg[:, 6:7],
                                        in1=stg[:, 2:3], op=OP.mult)
                nc.vector.tensor_tensor(out=stg[:, 7:8], in0=bnb_t[:, l:l + 1],
                                        in1=stg[:, 7:8], op=OP.subtract)
                if cfg.DEBUG:
                    nc.sync.dma_start(dbg_outb[l], outb[:])
                    nc.sync.dma_start(dbg_stg[l], stg[:])
                nc.scalar.activation(hT[:], outb[:], AF.Relu,
                                     bias=stg[:, 7:8], scale=stg[:, 6:7])

                if l + 1 < L:
                    emit_transpose_store(l + 1)

            # ---------- global mean pool + output MLP ----------
            psp = psB.tile([G, CHUNK], f32, tag="psB")
            for b in range(nblk):
                bw = min(P, NS - b * P)
                pst = psT.tile([P, P], bf16, tag="psT")
                nc.tensor.transpose(pst[:bw, :P], hT[:, b * P:b * P + bw],
                                    ident[:])
                rm = workpool.tile([P, P], bf16, tag="rm")
                nc.vector.tensor_copy(rm[:bw, :], pst[:bw, :P])
                ind = hotpool.tile([P, G], bf16, tag="ind")
                nc.vector.tensor_scalar(out=ind[:bw, :], in0=iota_bf[:bw, :G],
                                        scalar1=gids_t[:bw, b:b + 1],
                                        scalar2=None, op0=OP.is_equal)
                nc.tensor.matmul(out=psp[:, :H], lhsT=ind[:bw, :],
                                 rhs=rm[:bw, :], start=(b == 0),
                                 stop=(b == nblk - 1))
            poolt = workpool.tile([G, H], f32, tag="poolt")
            nc.vector.tensor_copy(poolt[:], psp[:, :H])
            nc.sync.dma_start(pool_in[:], poolt[:])
            nc.gpsimd.collective_compute(
                "AllReduce", OP.add, replica_groups=[cores],
                ins=[pool_in[:]], outs=[pool_out[:]])
            poolg = workpool.tile([G, H], f32, tag="poolg")
            nc.sync.dma_start(poolg[:], pool_out[:])
            if cfg.DEBUG:
                nc.gpsimd.dma_start(dbg_pool[:], pool_out[:])

            pstT = psT.tile([P, G], f32, tag="psTf")
            nc.tensor.transpose(pstT[:, :G], poolg[:], identf[:])
            poolT = workpool.tile([P, G], f32, tag="poolT")
            nc.vector.tensor_copy(poolT[:], pstT[:, :G])

            psl = psB.tile([C, CHUNK], f32, tag="psB")
            nc.tensor.matmul(out=psl[:, :G], lhsT=w_out_t[:], rhs=poolT[:],
                             start=True, stop=True)
            logit = workpool.tile([C, G], f32, tag="logit")
            nc.vector.tensor_tensor(out=logit[:], in0=psl[:, :G], in1=invg_t[:],
                                    op=OP.mult)
            logit2 = workpool.tile([C, G], f32, tag="logit2")
            nc.scalar.activation(logit2[:], logit[:], AF.Sigmoid,
                                 bias=b_out_t[:, 0:1], scale=1.0)
            nc.sync.dma_start(out_d[:], logit2[:])

    # The bass_exec custom-call lowering re-serializes the (finalized,
    # immutable) BIR on every run_bass_kernel_spmd call (~0.19s for this
    # module). Memoize the serialization on this instance.
    _orig_tjb = nc.to_json_bytes
    _json_cache = []

    def _cached_tjb():
        if not _json_cache:
            _json_cache.append(_orig_tjb())
        return _json_cache[0]

    nc.to_json_bytes = _cached_tjb
    return nc


def _make_in_maps(cfg, plan, inputs):
    H, C, G, F, NS, R, L = cfg.H, cfg.C, cfg.G, cfg.F, cfg.NS, cfg.R, cfg.L
    S_total = plan["S_total"]
    OFF, TOT = _layout(cfg, S_total)
    x = np.asarray(inputs["x"], np.float32)
    batch = np.asarray(inputs["batch"])

    relw = np.empty((cfg.NMAT, P, H), BF16)
    rel_w = np.asarray(inputs["rel_w"], np.float32)
    root_w = np.asarray(inputs["root_w"], np.float32)
    for l in range(L):
        for r in range(R):
            relw[l * R + r] = rel_w[l, r].astype(BF16)
        relw[L * R + l] = root_w[l].astype(BF16)
    # [P, NMAT*H] partition-major, split into per-core partition slices
    relw_pT = np.ascontiguousarray(
        relw.transpose(1, 0, 2)).reshape(P, cfg.MATCOLS)

    bng = np.ascontiguousarray(np.asarray(inputs["bn_g"], np.float32).T)
    bnb = np.ascontiguousarray(np.asarray(inputs["bn_b"], np.float32).T)
    b_in = np.asarray(inputs["b_in"], np.float32).reshape(H, 1)
    b_out = np.asarray(inputs["b_out"], np.float32).reshape(C, 1)
    w_in = np.asarray(inputs["w_in"], np.float32).astype(BF16)
    w_out = np.asarray(inputs["w_out"], np.float32)
    invg = np.ascontiguousarray(np.broadcast_to(
        plan["inv_gcnt"].astype(np.float32)[None, :], (C, G)))

    def put(blob, name, arr):
        b = np.frombuffer(arr.tobytes(), np.uint8)
        blob[OFF[name]:OFF[name] + b.size] = b

    nblk = cfg.nblk
    in_maps = []
    for c in range(cfg.NC):
        lo, hi = c * NS, (c + 1) * NS
        xT8 = np.ascontiguousarray(x[lo:hi].T).astype(FP8)
        gids = np.full((P, nblk), 255, np.uint8)
        bseg = batch[lo:hi].astype(np.uint8)
        for b in range(nblk):
            bw = min(P, NS - b * P)
            gids[:bw, b] = bseg[b * P:b * P + bw]
        blob = np.zeros(TOT, np.uint8)
        put(blob, "x8", xT8)
        put(blob, "lo", np.ascontiguousarray(plan["loA"][c]))
        put(blob, "loc", np.ascontiguousarray(plan["locA"][c]))
        put(blob, "cnt", np.ascontiguousarray(plan["cntA"][c]))
        put(blob, "relw",
            np.ascontiguousarray(relw_pT[c * cfg.PSL:(c + 1) * cfg.PSL]))
        put(blob, "w_in", w_in)
        put(blob, "gids", gids)
        put(blob, "invg", invg)
        put(blob, "bng", bng)
        put(blob, "bnb", bnb)
        put(blob, "b_in", b_in)
        put(blob, "w_out", w_out)
        put(blob, "b_out", b_out)
        in_maps.append(dict(blob=blob))
    return in_maps


def _run(cfg, inputs, **kw):
    plan = _plan(cfg, np.asarray(inputs["edge_index"]),
                 np.asarray(inputs["edge_type"]), np.asarray(inputs["batch"]))
    nc = _build_nc(cfg, plan)
    if not nc.is_finalized():
        nc.finalize()
    in_maps = _make_in_maps(cfg, plan, inputs)
    res = run_bass_kernel_spmd(nc, in_maps, core_ids=list(range(cfg.NC)), **kw)
    out = res.results[0]["out"]
    return np.ascontiguousarray(np.asarray(out).T.astype(np.float32)), res


def kernel(**inputs):
    cfg = Cfg()
    out, _ = _run(cfg, inputs)
    return out


# revision 23
# speedup vs baseline: 1.0443x; 1.0443x over previous
"""Trainium2 Bass kernel for 2-layer RGCN (nn_PygModel_52003464020165).

Self-contained: accepts FULL inputs, shards across 8 NeuronCores internally,
returns FULL [64, 10] output.

Architecture (per core, dst-sharded graph):
  - ALL per-core inputs packed into ONE u8 blob (host->device transfer over
    the axon tunnel is the wall-clock bottleneck: ~35 MB/s + ~90ms/array).
    x ships as fp8(e4m3), edge slots as 5 bytes (u16+u8 src idx, u8 loc,
    u8 count -> alpha via device reciprocal), relation weights sharded
    across cores and AllGathered, iota/identity generated on device.
  - full h replicated each layer via AllGather (bf16, [N, H] row-major DRAM)
  - per dst-chunk (512 dense dst cols): batched indirect-DMA gather of
    h[src] rows -> msg tiles [128 edges, H] (edges on partitions)
  - per relation r: alpha-hot matrices [128 edges, 128 win] built by one DVE
    tensor_scalar (is_equal vs iota, scaled by 1/cnt); PE matmuls
    msg^T @ alphahot accumulate mean bins into PSUM [H, chunk]
  - transform: root matmul + 20 relation matmuls (W_r stationary, bf16)
    accumulate out^T [H, chunk] in PSUM; evacuation fuses BN partial stats
  - BatchNorm stats via AllReduce; affine+ReLU as one ACT op over [H, NS]
  - PE transposes h^T -> row-major shard -> DRAM -> AllGather
  - global mean pool via indicator matmuls + AllReduce; final linear+sigmoid
"""

import math
import sys

sys.path.insert(0, "/opt/trn_rl_repo")

import ml_dtypes
import numpy as np

# Persistent XLA compilation cache: run_bass_kernel_spmd re-jits a fresh
# closure on every call, so without this each call pays ~1.2s of XLA
# compile; with it the recompile is a ~30ms disk-cache hit.
try:
    import jax as _jax

    _jax.config.update("jax_compilation_cache_dir", "/tmp/jax_comp_cache")
    _jax.config.update("jax_persistent_cache_min_compile_time_secs", 0.0)
    _jax.config.update("jax_persistent_cache_min_entry_size_bytes", 0)
except Exception:
    pass

import concourse.bacc as bacc
import concourse.bass as bass
import concourse.tile as tile
from concourse import mybir
from concourse.bass_utils import run_bass_kernel_spmd

BF16 = ml_dtypes.bfloat16
FP8 = ml_dtypes.float8_e4m3
P = 128
ALIGN = 512


class Cfg:
    def __init__(self, N=100000, E=1600000, F=64, H=128, R=20, G=64, C=10, L=2,
                 NC=8, CHUNK=512, WIN=128, EPS=1e-5, DT="bf16", DEBUG=False):
        assert H == P
        self.N, self.E, self.F, self.H, self.R, self.G, self.C, self.L = (
            N, E, F, H, R, G, C, L)
        self.NC, self.CHUNK, self.WIN, self.EPS = NC, CHUNK, WIN, EPS
        self.DT = DT
        self.DEBUG = DEBUG
        assert N % NC == 0
        self.NS = N // NC
        self.nchunks = math.ceil(self.NS / CHUNK)
        self.cw = [min(CHUNK, self.NS - c * CHUNK) for c in range(self.nchunks)]
        self.nwin = [math.ceil(w / WIN) for w in self.cw]
        self.nblk = math.ceil(self.NS / P)
        # relation-weight stack: L*R rel mats + L root mats, padded so each
        # core ships an equal partition-slice
        self.NMAT = L * R + L
        self.MATCOLS = self.NMAT * H              # 42*128 = 5376
        self.PSL = P // NC                        # partition rows per core


def _aligned_layout(fields):
    """fields: list of (name, nbytes). Returns (offsets dict, total)."""
    off = {}
    cur = 0
    for name, nb in fields:
        cur = (cur + ALIGN - 1) // ALIGN * ALIGN
        off[name] = cur
        cur += nb
    total = (cur + ALIGN - 1) // ALIGN * ALIGN
    return off, total


def _layout(cfg, S_total):
    F, H, R, G, C, L, NS = cfg.F, cfg.H, cfg.R, cfg.G, cfg.C, cfg.L, cfg.NS
    fields = [
        ("x8", F * NS),                 # fp8 [F, NS]
        ("lo", P * S_total * 2),        # u16 [P, S] low 16 idx bits
        ("loc", P * S_total),           # u8 [P, S]: loc | (idx_hi << 7)
        # in-degree nibbles (cnt < 16), 0 = sentinel; two slots per byte
        ("cnt", P * ((S_total + 1) // 2)),
        ("relw", cfg.PSL * cfg.MATCOLS * 2),  # bf16 [PSL, MATCOLS]
        ("w_in", F * H * 2),            # bf16 [F, H]
        ("gids", P * cfg.nblk),         # u8 [P, nblk]
        ("invg", C * G * 4),            # f32 [C, G]
        ("bng", H * L * 4),             # f32 [H, L]
        ("bnb", H * L * 4),             # f32 [H, L]
        ("b_in", H * 4),                # f32 [H, 1]
        ("w_out", H * C * 4),           # f32 [H, C]
        ("b_out", C * 4),               # f32 [C, 1]
    ]
    return _aligned_layout(fields)


def _plan(cfg, edge_index, edge_type, batch):
    """Host-side planner. Returns shared structure + per-core data arrays."""
    N, R, NC, NS, CHUNK, WIN = cfg.N, cfg.R, cfg.NC, cfg.NS, cfg.CHUNK, cfg.WIN
    src = edge_index[0].astype(np.int64)
    dst = edge_index[1].astype(np.int64)
    et = edge_type.astype(np.int64)

    comb = dst * R + et
    cnt = np.bincount(comb, minlength=N * R)
    cnt_e = np.maximum(cnt[comb], 1)
    assert cnt_e.max() < 16

    core = dst // NS
    dloc = dst % NS
    chunk = dloc // CHUNK
    inchunk = dloc % CHUNK
    win = inchunk // WIN
    loc = (inchunk % WIN).astype(np.int64)

    maxwin = max(cfg.nwin)
    gid = (chunk * R + et) * maxwin + win
    ngroups = cfg.nchunks * R * maxwin

    counts = np.zeros((NC, ngroups), np.int64)
    np.add.at(counts, (core, gid), 1)
    Tg = np.maximum(1, -(-counts.max(axis=0) // P))  # ceil div, min 1

    # tile order: chunk-major, then r, then win
    group_order = []
    for c in range(cfg.nchunks):
        for r in range(R):
            for w in range(cfg.nwin[c]):
                group_order.append((c * R + r) * maxwin + w)
    group_order = np.array(group_order, np.int64)
    tiles_of_group = Tg[group_order]
    tile_base = np.zeros(len(group_order), np.int64)
    np.cumsum(tiles_of_group[:-1], out=tile_base[1:])
    S_total = int(tiles_of_group.sum())

    gpos = np.full(ngroups, -1, np.int64)
    gpos[group_order] = np.arange(len(group_order))

    locA = np.zeros((NC, P, S_total), np.uint8)  # loc | (idx_hi << 7)
    cntA = np.zeros((NC, P, S_total), np.uint8)  # 0 = sentinel (alpha -> 0)
    srcA = np.zeros((NC, P, S_total), np.int64)  # sentinel: gather row 0

    order = np.lexsort((gid, core))
    s_core, s_gid = core[order], gid[order]
    s_src, s_loc, s_cnt = src[order], loc[order], cnt_e[order]
    key = s_core * ngroups + s_gid
    first = np.r_[True, key[1:] != key[:-1]]
    grp_start = np.flatnonzero(first)
    seglen = np.diff(np.r_[grp_start, len(key)])
    rank = np.arange(len(key)) - np.repeat(grp_start, seglen)

    slot = tile_base[gpos[s_gid]] * P + rank
    srcA[s_core, slot % P, slot // P] = s_src
    locA[s_core, slot % P, slot // P] = s_loc + 128 * (s_src >> 16)
    cntA[s_core, slot % P, slot // P] = s_cnt

    # emission structure: per chunk -> (slot_lo, slot_hi,
    #   per-r list of per-win (tile_base, ntiles))
    chunk_tiles = []
    for c in range(cfg.nchunks):
        lo = None
        hi = 0
        rlists = []
        for r in range(R):
            wl = []
            for w in range(cfg.nwin[c]):
                pos = gpos[(c * R + r) * maxwin + w]
                tb, tn = int(tile_base[pos]), int(tiles_of_group[pos])
                if lo is None:
                    lo = tb
                hi = tb + tn
                wl.append((tb, tn, w))
            rlists.append(wl)
        chunk_tiles.append((lo, hi, rlists))

    loA = (srcA & 0xFFFF).astype(np.uint16)
    assert srcA.max() < (1 << 17)

    gcnt = np.bincount(batch.astype(np.int64), minlength=cfg.G).astype(np.float32)
    inv_gcnt = 1.0 / np.maximum(gcnt, 1.0)

    return dict(S_total=S_total, chunk_tiles=chunk_tiles, locA=locA,
                cntA=cntA, loA=loA, inv_gcnt=inv_gcnt)


def _build_nc(cfg, plan):
    """Emit the SPMD Bass program (one program, NC cores)."""
    N, F, H, R, G, C, L = cfg.N, cfg.F, cfg.H, cfg.R, cfg.G, cfg.C, cfg.L
    NS, CHUNK, WIN = cfg.NS, cfg.CHUNK, cfg.WIN
    S_total = plan["S_total"]
    chunk_tiles = plan["chunk_tiles"]
    nblk = cfg.nblk
    OFF, TOT = _layout(cfg, S_total)

    nc = bacc.Bacc(None)
    f32, i32 = mybir.dt.float32, mybir.dt.int32
    u8, u16 = mybir.dt.uint8, mybir.dt.uint16
    bf16 = mybir.dt.bfloat16
    f8 = mybir.dt.float8e4
    AF = mybir.ActivationFunctionType
    OP = mybir.AluOpType
    assert cfg.DT == "bf16"

    blob_d = nc.dram_tensor("blob", [TOT], u8, kind="ExternalInput")
    out_d = nc.dram_tensor("out", [C, G], f32, kind="ExternalOutput")

    def v(name, shape, dt_):
        nbytes = math.prod(shape) * mybir.dt.size(dt_)
        ap = blob_d[OFF[name]:OFF[name] + nbytes].bitcast(dt_)
        if len(shape) == 2:
            ap = ap.rearrange("(a b) -> a b", b=shape[1])
        return ap

    relw_in = nc.dram_tensor("relw_in", [cfg.PSL, cfg.MATCOLS], bf16)
    relw_full = nc.dram_tensor("relw_full", [P, cfg.MATCOLS], bf16,
                               addr_space="Shared")
    h_shard = [nc.dram_tensor(f"h_shard{l}", [NS, H], bf16) for l in range(L)]
    h_full = [nc.dram_tensor(f"h_full{l}", [N, H], bf16, addr_space="Shared")
              for l in range(L)]
    stats_in = nc.dram_tensor("stats_in", [H, 2], f32)
    stats_out = nc.dram_tensor("stats_out", [H, 2], f32, addr_space="Shared")
    pool_in = nc.dram_tensor("pool_in", [G, H], f32)
    pool_out = nc.dram_tensor("pool_out", [G, H], f32, addr_space="Shared")
    if cfg.DEBUG:
        dbg_h = [nc.dram_tensor(f"dbg_h{l}", [N, H], bf16,
                                kind="ExternalOutput") for l in range(L)]
        dbg_outb = nc.dram_tensor("dbg_outb", [L, H, NS], bf16,
                                  kind="ExternalOutput")
        dbg_stg = nc.dram_tensor("dbg_stg", [L, H, 8], f32,
                                 kind="ExternalOutput")
        dbg_pool = nc.dram_tensor("dbg_pool", [G, H], f32,
                                  kind="ExternalOutput")

    cores = list(range(cfg.NC))

    with tile.TileContext(nc) as tc:
        with (
            tc.tile_pool(name="const", bufs=1) as cpool,
            tc.tile_pool(name="big", bufs=1) as bigpool,
            tc.tile_pool(name="msg", bufs=2) as msgpool,
            tc.tile_pool(name="hot", bufs=4) as hotpool,
            tc.tile_pool(name="mean", bufs=2) as meanpool,
            tc.tile_pool(name="work", bufs=3) as workpool,
            tc.tile_pool(name="psA", bufs=2, space="PSUM") as psA,
            tc.tile_pool(name="psT", bufs=2, space="PSUM") as psT,
            tc.tile_pool(name="psB", bufs=2, space="PSUM") as psB,
        ):
            # ---------- relation-weight AllGather (tiny; kick off first) ----
            nc.sync.dma_start(relw_in[:], v("relw", [cfg.PSL, cfg.MATCOLS],
                                            bf16))
            nc.gpsimd.collective_compute(
                "AllGather", OP.bypass, replica_groups=[cores],
                ins=[relw_in[:]], outs=[relw_full[:]])
            relw_t = cpool.tile([P, cfg.NMAT, H], bf16, tag="relw")
            nc.sync.dma_start(
                relw_t[:], relw_full[:].rearrange("p (m h) -> p m h", h=H))

            # ---------- generated constants ----------
            iota_bf = cpool.tile([P, WIN], bf16, tag="iota_bf")
            nc.gpsimd.iota(iota_bf[:], pattern=[[1, WIN]], base=0,
                           channel_multiplier=0,
                           allow_small_or_imprecise_dtypes=True)
            idiag = cpool.tile([P, P], i32, tag="idiag")
            nc.gpsimd.iota(idiag[:], pattern=[[1, P]], base=0,
                           channel_multiplier=-1)
            ident = cpool.tile([P, P], bf16, tag="ident")
            nc.vector.tensor_scalar(out=ident[:], in0=idiag[:], scalar1=0,
                                    scalar2=None, op0=OP.is_equal)
            identf = cpool.tile([G, G], f32, tag="identf")
            nc.vector.tensor_scalar(out=identf[:], in0=idiag[:G, :G],
                                    scalar1=0, scalar2=None, op0=OP.is_equal)

            # ---------- edge-data unpack ----------
            # planes: lo = low 16 idx bits; lochi = loc | (idx_hi << 7);
            # cnt = in-degree with 0 as the padding sentinel
            lo_t = cpool.tile([P, S_total], u16, tag="lo")
            nc.sync.dma_start(lo_t[:], v("lo", [P, S_total], u16))
            loc8_t = cpool.tile([P, S_total], u8, tag="loc8")
            nc.sync.dma_start(loc8_t[:], v("loc", [P, S_total], u8))
            Sh = (S_total + 1) // 2
            cntn_t = cpool.tile([P, Sh], u8, tag="cntn")
            nc.sync.dma_start(cntn_t[:], v("cnt", [P, Sh], u8))

            loc_t = cpool.tile([P, S_total], f32, tag="loc")
            alp_t = cpool.tile([P, S_total], f32, tag="alp")
            scr_t = cpool.tile([P, S_total], f32, tag="scr")
            cntf_t = cpool.tile([P, 2 * Sh], f32, tag="cntf")
            idx_t = cpool.tile([P, 2 * Sh], i32, tag="idx")
            # cnt nibble unpack (idx_t doubles as i32 scratch pre-idx-build)
            nc.vector.tensor_copy(idx_t[:, :Sh], cntn_t[:])
            cv = cntf_t[:].rearrange("p (s two) -> p s two", two=2)
            nc.vector.tensor_scalar(out=idx_t[:, Sh:2 * Sh],
                                    in0=idx_t[:, :Sh], scalar1=15,
                                    scalar2=None, op0=OP.bitwise_and)
            nc.vector.tensor_copy(cv[:, :, 0:1],
                                  idx_t[:, Sh:2 * Sh].unsqueeze(2))
            nc.vector.tensor_scalar(out=idx_t[:, Sh:2 * Sh],
                                    in0=idx_t[:, :Sh], scalar1=4,
                                    scalar2=None, op0=OP.logical_shift_right)
            nc.vector.tensor_copy(cv[:, :, 1:2],
                                  idx_t[:, Sh:2 * Sh].unsqueeze(2))
            # (all arithmetic exact in f32: values < 2^17)
            nc.vector.tensor_copy(scr_t[:], loc8_t[:])          # byte
            nc.vector.tensor_scalar(out=alp_t[:], in0=scr_t[:],
                                    scalar1=128.0, scalar2=None,
                                    op0=OP.is_ge)               # idx_hi
            nc.vector.tensor_copy(loc_t[:], lo_t[:])            # idx_lo
            nc.vector.tensor_scalar(out=alp_t[:], in0=alp_t[:],
                                    scalar1=65536.0, scalar2=None,
                                    op0=OP.mult)                # idx_hi<<16
            nc.vector.tensor_tensor(out=loc_t[:], in0=loc_t[:], in1=alp_t[:],
                                    op=OP.add)                  # idx
            nc.vector.tensor_copy(idx_t[:, :S_total], loc_t[:])
            nc.vector.tensor_scalar(out=alp_t[:], in0=alp_t[:],
                                    scalar1=1.0 / 512.0, scalar2=None,
                                    op0=OP.mult)                # idx_hi<<7
            nc.vector.tensor_tensor(out=loc_t[:], in0=scr_t[:], in1=alp_t[:],
                                    op=OP.subtract)             # loc
            nc.vector.tensor_scalar(out=scr_t[:], in0=cntf_t[:, :S_total],
                                    scalar1=0.0, scalar2=None,
                                    op0=OP.is_gt)               # pad mask
            nc.vector.tensor_scalar(out=alp_t[:], in0=cntf_t[:, :S_total],
                                    scalar1=1.0, scalar2=None,
                                    op0=OP.max)
            nc.vector.reciprocal(alp_t[:], alp_t[:])
            nc.vector.tensor_tensor(out=alp_t[:], in0=alp_t[:], in1=scr_t[:],
                                    op=OP.mult)                 # alpha

            # ---------- small params ----------
            w_in_t = cpool.tile([F, H], bf16, tag="w_in")
            nc.sync.dma_start(w_in_t[:], v("w_in", [F, H], bf16))
            b_in_t = cpool.tile([H, 1], f32, tag="b_in")
            nc.sync.dma_start(b_in_t[:], v("b_in", [H, 1], f32))
            bng_t = cpool.tile([H, L], f32, tag="bng")
            nc.sync.dma_start(bng_t[:], v("bng", [H, L], f32))
            bnb_t = cpool.tile([H, L], f32, tag="bnb")
            nc.sync.dma_start(bnb_t[:], v("bnb", [H, L], f32))
            gids8_t = cpool.tile([P, nblk], u8, tag="gids8")
            nc.sync.dma_start(gids8_t[:], v("gids", [P, nblk], u8))
            gids_t = cpool.tile([P, nblk], f32, tag="gids")
            nc.vector.tensor_copy(gids_t[:], gids8_t[:])
            w_out_t = cpool.tile([H, C], f32, tag="w_out")
            nc.sync.dma_start(w_out_t[:], v("w_out", [H, C], f32))
            b_out_t = cpool.tile([C, 1], f32, tag="b_out")
            nc.sync.dma_start(b_out_t[:], v("b_out", [C, 1], f32))
            invg_t = cpool.tile([C, G], f32, tag="invg")
            nc.sync.dma_start(invg_t[:], v("invg", [C, G], f32))

            # fences: pull const-load DMA completions into engine program
            # order one DMA at a time, so compute ops (tiny ISA wait
            # budgets) emit no DMA waits of their own
            fence = cpool.tile([1, 1], f32, tag="fence")
            for _ft in (bng_t, bnb_t, invg_t, w_out_t):
                nc.vector.tensor_copy(fence[:], _ft[0:1, 0:1])
            fenceA = cpool.tile([1, 1], f32, tag="fenceA")
            for _ft in (b_in_t, b_out_t):
                nc.scalar.copy(fenceA[:], _ft[0:1, 0:1])
            nc.scalar.copy(fenceA[:], relw_t[0:1, 0, 0:2].bitcast(f32))
            nc.scalar.copy(fenceA[:], w_in_t[0:1, 0:2].bitcast(f32))

            hT = bigpool.tile([P, NS], bf16, tag="hT")
            outb = bigpool.tile([P, NS], bf16, tag="outb")
            sum_parts = bigpool.tile([P, cfg.nchunks], f32, tag="sumP")
            sq_parts = bigpool.tile([P, cfg.nchunks], f32, tag="sqP")
            sq_scr = bigpool.tile([P, CHUNK], bf16, tag="sqscr")

            # ---------- input MLP ----------
            xv = v("x8", [F, NS], f8)
            for c in range(cfg.nchunks):
                cw = cfg.cw[c]
                xc8 = workpool.tile([F, CHUNK], f8, tag="xc8")
                nc.sync.dma_start(xc8[:, :cw], xv[:, c * CHUNK:c * CHUNK + cw])
                xc = workpool.tile([F, CHUNK], bf16, tag="xc")
                nc.vector.tensor_copy(xc[:, :cw], xc8[:, :cw])
                ps = psB.tile([P, CHUNK], f32, tag="psB")
                nc.tensor.matmul(out=ps[:, :cw], lhsT=w_in_t[:], rhs=xc[:, :cw],
                                 start=True, stop=True)
                nc.scalar.activation(hT[:, c * CHUNK:c * CHUNK + cw], ps[:, :cw],
                                     AF.Relu, bias=b_in_t[:, 0:1], scale=1.0)

            def emit_transpose_store(l):
                for b in range(nblk):
                    bw = min(P, NS - b * P)
                    pst = psT.tile([P, P], bf16, tag="psT")
                    nc.tensor.transpose(pst[:bw, :P], hT[:, b * P:b * P + bw],
                                        ident[:])
                    rm = workpool.tile([P, P], bf16, tag="rm")
                    nc.vector.tensor_copy(rm[:bw, :], pst[:bw, :P])
                    nc.sync.dma_start(h_shard[l][b * P:b * P + bw, :], rm[:bw, :])
                nc.gpsimd.collective_compute(
                    "AllGather", OP.bypass, replica_groups=[cores],
                    ins=[h_shard[l][:]], outs=[h_full[l][:]])
                if cfg.DEBUG:
                    nc.gpsimd.dma_start(dbg_h[l][:], h_full[l][:])

            emit_transpose_store(0)

            # ---------- RGCN layers ----------
            for l in range(L):
                root_i = L * R + l
                for c in range(cfg.nchunks):
                    cw = cfg.cw[c]
                    lo, hi, rlists = chunk_tiles[c]
                    nS = hi - lo
                    msg = msgpool.tile([P, nS, H], bf16, tag="msg")
                    for s in range(nS):
                        nc.gpsimd.indirect_dma_start(
                            out=msg[:, s, :], out_offset=None,
                            in_=h_full[l][:],
                            in_offset=bass.IndirectOffsetOnAxis(
                                ap=idx_t[:, lo + s:lo + s + 1], axis=0))

                    mean = meanpool.tile([P, R, CHUNK], bf16, tag="mean")
                    for r in range(R):
                        psa = psA.tile([P, CHUNK], f32, tag="psA")
                        for (tb, tn, w) in rlists[r]:
                            ww = min(WIN, cw - w * WIN)
                            for t in range(tn):
                                s = tb + t
                                hot = hotpool.tile([P, WIN], bf16, tag="hot")
                                nc.vector.tensor_scalar(
                                    out=hot[:, :ww], in0=iota_bf[:, :ww],
                                    scalar1=loc_t[:, s:s + 1],
                                    scalar2=alp_t[:, s:s + 1],
                                    op0=OP.is_equal, op1=OP.mult)
                                nc.tensor.matmul(
                                    out=psa[:, w * WIN:w * WIN + ww],
                                    lhsT=msg[:, s - lo, :], rhs=hot[:, :ww],
                                    start=(t == 0), stop=(t == tn - 1))
                        if r % 2 == 0:
                            nc.vector.tensor_copy(mean[:, r, :cw], psa[:, :cw])
                        else:
                            nc.scalar.copy(mean[:, r, :cw], psa[:, :cw])

                    psb = psB.tile([P, CHUNK], f32, tag="psB")
                    nc.tensor.matmul(out=psb[:, :cw], lhsT=relw_t[:, root_i, :],
                                     rhs=hT[:, c * CHUNK:c * CHUNK + cw],
                                     start=True, stop=False)
                    for r in range(R):
                        nc.tensor.matmul(out=psb[:, :cw],
                                         lhsT=relw_t[:, l * R + r, :],
                                         rhs=mean[:, r, :cw],
                                         start=False, stop=(r == R - 1))

                    nc.vector.tensor_scalar(
                        out=outb[:, c * CHUNK:c * CHUNK + cw], in0=psb[:, :cw],
                        scalar1=1.0, scalar2=None, op0=OP.mult, op1=OP.add,
                        accum_out=sum_parts[:, c:c + 1])
                    nc.scalar.activation(sq_scr[:, :cw], psb[:, :cw], AF.Square,
                                         accum_out=sq_parts[:, c:c + 1])

                # ---------- BatchNorm + ReLU ----------
                st = workpool.tile([H, 2], f32, tag="stats")
                nc.vector.reduce_sum(st[:, 0:1], sum_parts[:],
                                     axis=mybir.AxisListType.X)
                nc.vector.reduce_sum(st[:, 1:2], sq_parts[:],
                                     axis=mybir.AxisListType.X)
                nc.sync.dma_start(stats_in[:], st[:])
                nc.gpsimd.collective_compute(
                    "AllReduce", OP.add, replica_groups=[cores],
                    ins=[stats_in[:]], outs=[stats_out[:]])
                stg = workpool.tile([H, 8], f32, tag="stg")
                nc.sync.dma_start(stg[:, 0:2], stats_out[:])
                nc.vector.tensor_scalar(out=stg[:, 2:3], in0=stg[:, 0:1],
                                        scalar1=1.0 / N, scalar2=None,
                                        op0=OP.mult)
                nc.vector.tensor_scalar(out=stg[:, 3:4], in0=stg[:, 1:2],
                                        scalar1=1.0 / N, scalar2=None,
                                        op0=OP.mult)
                nc.vector.tensor_tensor(out=stg[:, 4:5], in0=stg[:, 2:3],
                                        in1=stg[:, 2:3], op=OP.mult)
                nc.vector.tensor_tensor(out=stg[:, 4:5], in0=stg[:, 3:4],
                                        in1=stg[:, 4:5], op=OP.subtract)
                nc.vector.tensor_scalar(out=stg[:, 4:5], in0=stg[:, 4:5],
                                        scalar1=cfg.EPS, scalar2=None,
                                        op0=OP.add)
                nc.scalar.sqrt(stg[:, 5:6], stg[:, 4:5])
                nc.vector.reciprocal(stg[:, 6:7], stg[:, 5:6])
                nc.vector.tensor_tensor(out=stg[:, 6:7], in0=stg[:, 6:7],
                                        in1=bng_t[:, l:l + 1], op=OP.mult)
                nc.vector.tensor_tensor(out=stg[:, 7:8], in0=stg[:, 6:7],
                                        in1=stg[:, 2:3], op=OP.mult)
                nc.vector.tensor_tensor(out=stg[:, 7:8], in0=bnb_t[:, l:l + 1],
                                        in1=stg[:, 7:8], op=OP.subtract)
                if cfg.DEBUG:
                    nc.sync.dma_start(dbg_outb[l], outb[:])
                    nc.sync.dma_start(dbg_stg[l], stg[:])
                nc.scalar.activation(hT[:], outb[:], AF.Relu,
                                     bias=stg[:, 7:8], scale=stg[:, 6:7])

                if l + 1 < L:
                    emit_transpose_store(l + 1)

            # ---------- global mean pool + output MLP ----------
            psp = psB.tile([G, CHUNK], f32, tag="psB")
            for b in range(nblk):
                bw = min(P, NS - b * P)
                pst = psT.tile([P, P], bf16, tag="psT")
                nc.tensor.transpose(pst[:bw, :P], hT[:, b * P:b * P + bw],
                                    ident[:])
                rm = workpool.tile([P, P], bf16, tag="rm")
                nc.vector.tensor_copy(rm[:bw, :], pst[:bw, :P])
                ind = hotpool.tile([P, G], bf16, tag="ind")
                nc.vector.tensor_scalar(out=ind[:bw, :], in0=iota_bf[:bw, :G],
                                        scalar1=gids_t[:bw, b:b + 1],
                                        scalar2=None, op0=OP.is_equal)
                nc.tensor.matmul(out=psp[:, :H], lhsT=ind[:bw, :],
                                 rhs=rm[:bw, :], start=(b == 0),
                                 stop=(b == nblk - 1))
            poolt = workpool.tile([G, H], f32, tag="poolt")
            nc.vector.tensor_copy(poolt[:], psp[:, :H])
            nc.sync.dma_start(pool_in[:], poolt[:])
            nc.gpsimd.collective_compute(
                "AllReduce", OP.add, replica_groups=[cores],
                ins=[pool_in[:]], outs=[pool_out[:]])
            poolg = workpool.tile([G, H], f32, tag="poolg")
            nc.sync.dma_start(poolg[:], pool_out[:])
            if cfg.DEBUG:
                nc.gpsimd.dma_start(dbg_pool[:], pool_out[:])

            pstT = psT.tile([P, G], f32, tag="psTf")
            nc.tensor.transpose(pstT[:, :G], poolg[:], identf[:])
            poolT = workpool.tile([P, G], f32, tag="poolT")
            nc.vector.tensor_copy(poolT[:], pstT[:, :G])

            psl = psB.tile([C, CHUNK], f32, tag="psB")
            nc.tensor.matmul(out=psl[:, :G], lhsT=w_out_t[:], rhs=poolT[:],
                             start=True, stop=True)
            logit = workpool.tile([C, G], f32, tag="logit")
            nc.vector.tensor_tensor(out=logit[:], in0=psl[:, :G], in1=invg_t[:],
                                    op=OP.mult)
            logit2 = workpool.tile([C, G], f32, tag="logit2")
            nc.scalar.activation(logit2[:], logit[:], AF.Sigmoid,
                                 bias=b_out_t[:, 0:1], scale=1.0)
            nc.sync.dma_start(out_d[:], logit2[:])

    # The bass_exec custom-call lowering re-serializes the (finalized,
    # immutable) BIR on every run_bass_kernel_spmd call (~0.19s for this
    # module). Memoize the serialization on this instance.
    _orig_tjb = nc.to_json_bytes
    _json_cache = []

    def _cached_tjb():
        if not _json_cache:
            _json_cache.append(_orig_tjb())
        return _json_cache[0]

    nc.to_json_bytes = _cached_tjb
    return nc


def _make_in_maps(cfg, plan, inputs):
    H, C, G, F, NS, R, L = cfg.H, cfg.C, cfg.G, cfg.F, cfg.NS, cfg.R, cfg.L
    S_total = plan["S_total"]
    OFF, TOT = _layout(cfg, S_total)
    x = np.asarray(inputs["x"], np.float32)
    batch = np.asarray(inputs["batch"])

    relw = np.empty((cfg.NMAT, P, H), BF16)
    rel_w = np.asarray(inputs["rel_w"], np.float32)
    root_w = np.asarray(inputs["root_w"], np.float32)
    for l in range(L):
        for r in range(R):
            relw[l * R + r] = rel_w[l, r].astype(BF16)
        relw[L * R + l] = root_w[l].astype(BF16)
    # [P, NMAT*H] partition-major, split into per-core partition slices
    relw_pT = np.ascontiguousarray(
        relw.transpose(1, 0, 2)).reshape(P, cfg.MATCOLS)

    bng = np.ascontiguousarray(np.asarray(inputs["bn_g"], np.float32).T)
    bnb = np.ascontiguousarray(np.asarray(inputs["bn_b"], np.float32).T)
    b_in = np.asarray(inputs["b_in"], np.float32).reshape(H, 1)
    b_out = np.asarray(inputs["b_out"], np.float32).reshape(C, 1)
    w_in = np.asarray(inputs["w_in"], np.float32).astype(BF16)
    w_out = np.asarray(inputs["w_out"], np.float32)
    invg = np.ascontiguousarray(np.broadcast_to(
        plan["inv_gcnt"].astype(np.float32)[None, :], (C, G)))

    def put(blob, name, arr):
        b = np.frombuffer(arr.tobytes(), np.uint8)
        blob[OFF[name]:OFF[name] + b.size] = b

    nblk = cfg.nblk
    in_maps = []
    for c in range(cfg.NC):
        lo, hi = c * NS, (c + 1) * NS
        xT8 = np.ascontiguousarray(x[lo:hi].T).astype(FP8)
        gids = np.full((P, nblk), 255, np.uint8)
        bseg = batch[lo:hi].astype(np.uint8)
        for b in range(nblk):
            bw = min(P, NS - b * P)
            gids[:bw, b] = bseg[b * P:b * P + bw]
        blob = np.zeros(TOT, np.uint8)
        Sh = (S_total + 1) // 2
        cnt_pad = np.zeros((P, 2 * Sh), np.uint8)
        cnt_pad[:, :S_total] = plan["cntA"][c]
        cnt_nib = cnt_pad[:, 0::2] | (cnt_pad[:, 1::2] << 4)
        put(blob, "x8", xT8)
        put(blob, "lo", np.ascontiguousarray(plan["loA"][c]))
        put(blob, "loc", np.ascontiguousarray(plan["locA"][c]))
        put(blob, "cnt", np.ascontiguousarray(cnt_nib))
        put(blob, "relw",
            np.ascontiguousarray(relw_pT[c * cfg.PSL:(c + 1) * cfg.PSL]))
        put(blob, "w_in", w_in)
        put(blob, "gids", gids)
        put(blob, "invg", invg)
        put(blob, "bng", bng)
        put(blob, "bnb", bnb)
        put(blob, "b_in", b_in)
        put(blob, "w_out", w_out)
        put(blob, "b_out", b_out)
        in_maps.append(dict(blob=blob))
    return in_maps


def _run(cfg, inputs, **kw):
    plan = _plan(cfg, np.asarray(inputs["edge_index"]),
                 np.asarray(inputs["edge_type"]), np.asarray(inputs["batch"]))
    nc = _build_nc(cfg, plan)
    if not nc.is_finalized():
        nc.finalize()
    in_maps = _make_in_maps(cfg, plan, inputs)
    res = run_bass_kernel_spmd(nc, in_maps, core_ids=list(range(cfg.NC)), **kw)
    out = res.results[0]["out"]
    return np.ascontiguousarray(np.asarray(out).T.astype(np.float32)), res


def kernel(**inputs):
    cfg = Cfg()
    out, _ = _run(cfg, inputs)
    return out


# revision 25
# speedup vs baseline: 1.0707x; 1.0253x over previous
"""Trainium2 Bass kernel for 2-layer RGCN (nn_PygModel_52003464020165).

Self-contained: accepts FULL inputs, shards across 8 NeuronCores internally,
returns FULL [64, 10] output.

Architecture (per core, dst-sharded graph):
  - ALL per-core inputs packed into ONE u8 blob (host->device transfer over
    the axon tunnel is the wall-clock bottleneck: ~35 MB/s + ~90ms/array).
    x ships as fp8(e4m3), edge slots as 5 bytes (u16+u8 src idx, u8 loc,
    u8 count -> alpha via device reciprocal), relation weights sharded
    across cores and AllGathered, iota/identity generated on device.
  - full h replicated each layer via AllGather (bf16, [N, H] row-major DRAM)
  - per dst-chunk (512 dense dst cols): batched indirect-DMA gather of
    h[src] rows -> msg tiles [128 edges, H] (edges on partitions)
  - per relation r: alpha-hot matrices [128 edges, 128 win] built by one DVE
    tensor_scalar (is_equal vs iota, scaled by 1/cnt); PE matmuls
    msg^T @ alphahot accumulate mean bins into PSUM [H, chunk]
  - transform: root matmul + 20 relation matmuls (W_r stationary, bf16)
    accumulate out^T [H, chunk] in PSUM; evacuation fuses BN partial stats
  - BatchNorm stats via AllReduce; affine+ReLU as one ACT op over [H, NS]
  - PE transposes h^T -> row-major shard -> DRAM -> AllGather
  - global mean pool via indicator matmuls + AllReduce; final linear+sigmoid
"""

import math
import sys

sys.path.insert(0, "/opt/trn_rl_repo")

import ml_dtypes
import numpy as np

# Persistent XLA compilation cache: run_bass_kernel_spmd re-jits a fresh
# closure on every call, so without this each call pays ~1.2s of XLA
# compile; with it the recompile is a ~30ms disk-cache hit.
try:
    import jax as _jax

    _jax.config.update("jax_compilation_cache_dir", "/tmp/jax_comp_cache")
    _jax.config.update("jax_persistent_cache_min_compile_time_secs", 0.0)
    _jax.config.update("jax_persistent_cache_min_entry_size_bytes", 0)
except Exception:
    pass
# bass_exec normally declares a JAX effect, which forces the slow Python
# dispatch path (runtime-token bookkeeping per device per call). The
# effect exists only to surface device errors on never-read outputs;
# run_bass_kernel_spmd reads every output, so suppress it and take the
# C++ fast-path dispatch.
try:
    import concourse.bass2jax  # noqa: F401  (registers the config state)

    _jax.config.update("bass_fast_dispatch", True)
except Exception:
    pass

import concourse.bacc as bacc
import concourse.bass as bass
import concourse.tile as tile
from concourse import mybir
from concourse.bass_utils import run_bass_kernel_spmd

BF16 = ml_dtypes.bfloat16
FP8 = ml_dtypes.float8_e4m3
P = 128
ALIGN = 512


class Cfg:
    def __init__(self, N=100000, E=1600000, F=64, H=128, R=20, G=64, C=10, L=2,
                 NC=8, CHUNK=512, WIN=128, EPS=1e-5, DT="bf16", DEBUG=False):
        assert H == P
        self.N, self.E, self.F, self.H, self.R, self.G, self.C, self.L = (
            N, E, F, H, R, G, C, L)
        self.NC, self.CHUNK, self.WIN, self.EPS = NC, CHUNK, WIN, EPS
        self.DT = DT
        self.DEBUG = DEBUG
        assert N % NC == 0
        self.NS = N // NC
        self.nchunks = math.ceil(self.NS / CHUNK)
        self.cw = [min(CHUNK, self.NS - c * CHUNK) for c in range(self.nchunks)]
        self.nwin = [math.ceil(w / WIN) for w in self.cw]
        self.nblk = math.ceil(self.NS / P)
        # relation-weight stack: L*R rel mats + L root mats, padded so each
        # core ships an equal partition-slice
        self.NMAT = L * R + L
        self.MATCOLS = self.NMAT * H              # 42*128 = 5376
        self.PSL = P // NC                        # partition rows per core


def _aligned_layout(fields):
    """fields: list of (name, nbytes). Returns (offsets dict, total)."""
    off = {}
    cur = 0
    for name, nb in fields:
        cur = (cur + ALIGN - 1) // ALIGN * ALIGN
        off[name] = cur
        cur += nb
    total = (cur + ALIGN - 1) // ALIGN * ALIGN
    return off, total


def _layout(cfg, S_total):
    F, H, R, G, C, L, NS = cfg.F, cfg.H, cfg.R, cfg.G, cfg.C, cfg.L, cfg.NS
    fields = [
        ("x8", F * NS),                 # fp8 [F, NS]
        ("lo", P * S_total * 2),        # u16 [P, S] low 16 idx bits
        ("loc", P * S_total),           # u8 [P, S]: loc | (idx_hi << 7)
        # in-degree nibbles (cnt < 16), 0 = sentinel; two slots per byte
        ("cnt", P * ((S_total + 1) // 2)),
        ("relw", cfg.PSL * cfg.MATCOLS * 2),  # bf16 [PSL, MATCOLS]
        ("w_in", F * H * 2),            # bf16 [F, H]
        ("gids", P * cfg.nblk),         # u8 [P, nblk]
        ("invg", C * G * 4),            # f32 [C, G]
        ("bng", H * L * 4),             # f32 [H, L]
        ("bnb", H * L * 4),             # f32 [H, L]
        ("b_in", H * 4),                # f32 [H, 1]
        ("w_out", H * C * 4),           # f32 [H, C]
        ("b_out", C * 4),               # f32 [C, 1]
    ]
    return _aligned_layout(fields)


def _plan(cfg, edge_index, edge_type, batch):
    """Host-side planner. Returns shared structure + per-core data arrays."""
    N, R, NC, NS, CHUNK, WIN = cfg.N, cfg.R, cfg.NC, cfg.NS, cfg.CHUNK, cfg.WIN
    src = edge_index[0].astype(np.int64)
    dst = edge_index[1].astype(np.int64)
    et = edge_type.astype(np.int64)

    comb = dst * R + et
    cnt = np.bincount(comb, minlength=N * R)
    cnt_e = np.maximum(cnt[comb], 1)
    assert cnt_e.max() < 16

    core = dst // NS
    dloc = dst % NS
    chunk = dloc // CHUNK
    inchunk = dloc % CHUNK
    win = inchunk // WIN
    loc = (inchunk % WIN).astype(np.int64)

    maxwin = max(cfg.nwin)
    gid = (chunk * R + et) * maxwin + win
    ngroups = cfg.nchunks * R * maxwin

    counts = np.zeros((NC, ngroups), np.int64)
    np.add.at(counts, (core, gid), 1)
    Tg = np.maximum(1, -(-counts.max(axis=0) // P))  # ceil div, min 1

    # tile order: chunk-major, then r, then win
    group_order = []
    for c in range(cfg.nchunks):
        for r in range(R):
            for w in range(cfg.nwin[c]):
                group_order.append((c * R + r) * maxwin + w)
    group_order = np.array(group_order, np.int64)
    tiles_of_group = Tg[group_order]
    tile_base = np.zeros(len(group_order), np.int64)
    np.cumsum(tiles_of_group[:-1], out=tile_base[1:])
    S_total = int(tiles_of_group.sum())

    gpos = np.full(ngroups, -1, np.int64)
    gpos[group_order] = np.arange(len(group_order))

    locA = np.zeros((NC, P, S_total), np.uint8)  # loc | (idx_hi << 7)
    cntA = np.zeros((NC, P, S_total), np.uint8)  # 0 = sentinel (alpha -> 0)
    srcA = np.zeros((NC, P, S_total), np.int64)  # sentinel: gather row 0

    order = np.lexsort((gid, core))
    s_core, s_gid = core[order], gid[order]
    s_src, s_loc, s_cnt = src[order], loc[order], cnt_e[order]
    key = s_core * ngroups + s_gid
    first = np.r_[True, key[1:] != key[:-1]]
    grp_start = np.flatnonzero(first)
    seglen = np.diff(np.r_[grp_start, len(key)])
    rank = np.arange(len(key)) - np.repeat(grp_start, seglen)

    slot = tile_base[gpos[s_gid]] * P + rank
    srcA[s_core, slot % P, slot // P] = s_src
    locA[s_core, slot % P, slot // P] = s_loc + 128 * (s_src >> 16)
    cntA[s_core, slot % P, slot // P] = s_cnt

    # emission structure: per chunk -> (slot_lo, slot_hi,
    #   per-r list of per-win (tile_base, ntiles))
    chunk_tiles = []
    for c in range(cfg.nchunks):
        lo = None
        hi = 0
        rlists = []
        for r in range(R):
            wl = []
            for w in range(cfg.nwin[c]):
                pos = gpos[(c * R + r) * maxwin + w]
                tb, tn = int(tile_base[pos]), int(tiles_of_group[pos])
                if lo is None:
                    lo = tb
                hi = tb + tn
                wl.append((tb, tn, w))
            rlists.append(wl)
        chunk_tiles.append((lo, hi, rlists))

    loA = (srcA & 0xFFFF).astype(np.uint16)
    assert srcA.max() < (1 << 17)

    gcnt = np.bincount(batch.astype(np.int64), minlength=cfg.G).astype(np.float32)
    inv_gcnt = 1.0 / np.maximum(gcnt, 1.0)

    return dict(S_total=S_total, chunk_tiles=chunk_tiles, locA=locA,
                cntA=cntA, loA=loA, inv_gcnt=inv_gcnt)


def _build_nc(cfg, plan):
    """Emit the SPMD Bass program (one program, NC cores)."""
    N, F, H, R, G, C, L = cfg.N, cfg.F, cfg.H, cfg.R, cfg.G, cfg.C, cfg.L
    NS, CHUNK, WIN = cfg.NS, cfg.CHUNK, cfg.WIN
    S_total = plan["S_total"]
    chunk_tiles = plan["chunk_tiles"]
    nblk = cfg.nblk
    OFF, TOT = _layout(cfg, S_total)

    nc = bacc.Bacc(None)
    f32, i32 = mybir.dt.float32, mybir.dt.int32
    u8, u16 = mybir.dt.uint8, mybir.dt.uint16
    bf16 = mybir.dt.bfloat16
    f8 = mybir.dt.float8e4
    AF = mybir.ActivationFunctionType
    OP = mybir.AluOpType
    assert cfg.DT == "bf16"

    blob_d = nc.dram_tensor("blob", [TOT], u8, kind="ExternalInput")
    out_d = nc.dram_tensor("out", [C, G], f32, kind="ExternalOutput")

    def v(name, shape, dt_):
        nbytes = math.prod(shape) * mybir.dt.size(dt_)
        ap = blob_d[OFF[name]:OFF[name] + nbytes].bitcast(dt_)
        if len(shape) == 2:
            ap = ap.rearrange("(a b) -> a b", b=shape[1])
        return ap

    relw_in = nc.dram_tensor("relw_in", [cfg.PSL, cfg.MATCOLS], bf16)
    relw_full = nc.dram_tensor("relw_full", [P, cfg.MATCOLS], bf16,
                               addr_space="Shared")
    h_shard = [nc.dram_tensor(f"h_shard{l}", [NS, H], bf16) for l in range(L)]
    h_full = [nc.dram_tensor(f"h_full{l}", [N, H], bf16, addr_space="Shared")
              for l in range(L)]
    stats_in = nc.dram_tensor("stats_in", [H, 2], f32)
    stats_out = nc.dram_tensor("stats_out", [H, 2], f32, addr_space="Shared")
    pool_in = nc.dram_tensor("pool_in", [G, H], f32)
    pool_out = nc.dram_tensor("pool_out", [G, H], f32, addr_space="Shared")
    if cfg.DEBUG:
        dbg_h = [nc.dram_tensor(f"dbg_h{l}", [N, H], bf16,
                                kind="ExternalOutput") for l in range(L)]
        dbg_outb = nc.dram_tensor("dbg_outb", [L, H, NS], bf16,
                                  kind="ExternalOutput")
        dbg_stg = nc.dram_tensor("dbg_stg", [L, H, 8], f32,
                                 kind="ExternalOutput")
        dbg_pool = nc.dram_tensor("dbg_pool", [G, H], f32,
                                  kind="ExternalOutput")

    cores = list(range(cfg.NC))

    with tile.TileContext(nc) as tc:
        with (
            tc.tile_pool(name="const", bufs=1) as cpool,
            tc.tile_pool(name="big", bufs=1) as bigpool,
            tc.tile_pool(name="msg", bufs=2) as msgpool,
            tc.tile_pool(name="hot", bufs=4) as hotpool,
            tc.tile_pool(name="mean", bufs=2) as meanpool,
            tc.tile_pool(name="work", bufs=3) as workpool,
            tc.tile_pool(name="psA", bufs=2, space="PSUM") as psA,
            tc.tile_pool(name="psT", bufs=2, space="PSUM") as psT,
            tc.tile_pool(name="psB", bufs=2, space="PSUM") as psB,
        ):
            # ---------- relation-weight AllGather (tiny; kick off first) ----
            nc.sync.dma_start(relw_in[:], v("relw", [cfg.PSL, cfg.MATCOLS],
                                            bf16))
            nc.gpsimd.collective_compute(
                "AllGather", OP.bypass, replica_groups=[cores],
                ins=[relw_in[:]], outs=[relw_full[:]])
            relw_t = cpool.tile([P, cfg.NMAT, H], bf16, tag="relw")
            nc.sync.dma_start(
                relw_t[:], relw_full[:].rearrange("p (m h) -> p m h", h=H))

            # ---------- generated constants ----------
            iota_bf = cpool.tile([P, WIN], bf16, tag="iota_bf")
            nc.gpsimd.iota(iota_bf[:], pattern=[[1, WIN]], base=0,
                           channel_multiplier=0,
                           allow_small_or_imprecise_dtypes=True)
            idiag = cpool.tile([P, P], i32, tag="idiag")
            nc.gpsimd.iota(idiag[:], pattern=[[1, P]], base=0,
                           channel_multiplier=-1)
            ident = cpool.tile([P, P], bf16, tag="ident")
            nc.vector.tensor_scalar(out=ident[:], in0=idiag[:], scalar1=0,
                                    scalar2=None, op0=OP.is_equal)
            identf = cpool.tile([G, G], f32, tag="identf")
            nc.vector.tensor_scalar(out=identf[:], in0=idiag[:G, :G],
                                    scalar1=0, scalar2=None, op0=OP.is_equal)

            # ---------- edge-data unpack ----------
            # planes: lo = low 16 idx bits; lochi = loc | (idx_hi << 7);
            # cnt = in-degree with 0 as the padding sentinel
            lo_t = cpool.tile([P, S_total], u16, tag="lo")
            nc.sync.dma_start(lo_t[:], v("lo", [P, S_total], u16))
            loc8_t = cpool.tile([P, S_total], u8, tag="loc8")
            nc.sync.dma_start(loc8_t[:], v("loc", [P, S_total], u8))
            Sh = (S_total + 1) // 2
            cntn_t = cpool.tile([P, Sh], u8, tag="cntn")
            nc.sync.dma_start(cntn_t[:], v("cnt", [P, Sh], u8))

            loc_t = cpool.tile([P, S_total], f32, tag="loc")
            alp_t = cpool.tile([P, S_total], f32, tag="alp")
            scr_t = cpool.tile([P, S_total], f32, tag="scr")
            cntf_t = cpool.tile([P, 2 * Sh], f32, tag="cntf")
            idx_t = cpool.tile([P, 2 * Sh], i32, tag="idx")
            # cnt nibble unpack (idx_t doubles as i32 scratch pre-idx-build)
            nc.vector.tensor_copy(idx_t[:, :Sh], cntn_t[:])
            cv = cntf_t[:].rearrange("p (s two) -> p s two", two=2)
            nc.vector.tensor_scalar(out=idx_t[:, Sh:2 * Sh],
                                    in0=idx_t[:, :Sh], scalar1=15,
                                    scalar2=None, op0=OP.bitwise_and)
            nc.vector.tensor_copy(cv[:, :, 0:1],
                                  idx_t[:, Sh:2 * Sh].unsqueeze(2))
            nc.vector.tensor_scalar(out=idx_t[:, Sh:2 * Sh],
                                    in0=idx_t[:, :Sh], scalar1=4,
                                    scalar2=None, op0=OP.logical_shift_right)
            nc.vector.tensor_copy(cv[:, :, 1:2],
                                  idx_t[:, Sh:2 * Sh].unsqueeze(2))
            # (all arithmetic exact in f32: values < 2^17)
            nc.vector.tensor_copy(scr_t[:], loc8_t[:])          # byte
            nc.vector.tensor_scalar(out=alp_t[:], in0=scr_t[:],
                                    scalar1=128.0, scalar2=None,
                                    op0=OP.is_ge)               # idx_hi
            nc.vector.tensor_copy(loc_t[:], lo_t[:])            # idx_lo
            nc.vector.tensor_scalar(out=alp_t[:], in0=alp_t[:],
                                    scalar1=65536.0, scalar2=None,
                                    op0=OP.mult)                # idx_hi<<16
            nc.vector.tensor_tensor(out=loc_t[:], in0=loc_t[:], in1=alp_t[:],
                                    op=OP.add)                  # idx
            nc.vector.tensor_copy(idx_t[:, :S_total], loc_t[:])
            nc.vector.tensor_scalar(out=alp_t[:], in0=alp_t[:],
                                    scalar1=1.0 / 512.0, scalar2=None,
                                    op0=OP.mult)                # idx_hi<<7
            nc.vector.tensor_tensor(out=loc_t[:], in0=scr_t[:], in1=alp_t[:],
                                    op=OP.subtract)             # loc
            nc.vector.tensor_scalar(out=scr_t[:], in0=cntf_t[:, :S_total],
                                    scalar1=0.0, scalar2=None,
                                    op0=OP.is_gt)               # pad mask
            nc.vector.tensor_scalar(out=alp_t[:], in0=cntf_t[:, :S_total],
                                    scalar1=1.0, scalar2=None,
                                    op0=OP.max)
            nc.vector.reciprocal(alp_t[:], alp_t[:])
            nc.vector.tensor_tensor(out=alp_t[:], in0=alp_t[:], in1=scr_t[:],
                                    op=OP.mult)                 # alpha

            # ---------- small params ----------
            w_in_t = cpool.tile([F, H], bf16, tag="w_in")
            nc.sync.dma_start(w_in_t[:], v("w_in", [F, H], bf16))
            b_in_t = cpool.tile([H, 1], f32, tag="b_in")
            nc.sync.dma_start(b_in_t[:], v("b_in", [H, 1], f32))
            bng_t = cpool.tile([H, L], f32, tag="bng")
            nc.sync.dma_start(bng_t[:], v("bng", [H, L], f32))
            bnb_t = cpool.tile([H, L], f32, tag="bnb")
            nc.sync.dma_start(bnb_t[:], v("bnb", [H, L], f32))
            gids8_t = cpool.tile([P, nblk], u8, tag="gids8")
            nc.sync.dma_start(gids8_t[:], v("gids", [P, nblk], u8))
            gids_t = cpool.tile([P, nblk], f32, tag="gids")
            nc.vector.tensor_copy(gids_t[:], gids8_t[:])
            w_out_t = cpool.tile([H, C], f32, tag="w_out")
            nc.sync.dma_start(w_out_t[:], v("w_out", [H, C], f32))
            b_out_t = cpool.tile([C, 1], f32, tag="b_out")
            nc.sync.dma_start(b_out_t[:], v("b_out", [C, 1], f32))
            invg_t = cpool.tile([C, G], f32, tag="invg")
            nc.sync.dma_start(invg_t[:], v("invg", [C, G], f32))

            # fences: pull const-load DMA completions into engine program
            # order one DMA at a time, so compute ops (tiny ISA wait
            # budgets) emit no DMA waits of their own
            fence = cpool.tile([1, 1], f32, tag="fence")
            for _ft in (bng_t, bnb_t, invg_t, w_out_t):
                nc.vector.tensor_copy(fence[:], _ft[0:1, 0:1])
            fenceA = cpool.tile([1, 1], f32, tag="fenceA")
            for _ft in (b_in_t, b_out_t):
                nc.scalar.copy(fenceA[:], _ft[0:1, 0:1])
            nc.scalar.copy(fenceA[:], relw_t[0:1, 0, 0:2].bitcast(f32))
            nc.scalar.copy(fenceA[:], w_in_t[0:1, 0:2].bitcast(f32))

            hT = bigpool.tile([P, NS], bf16, tag="hT")
            outb = bigpool.tile([P, NS], bf16, tag="outb")
            sum_parts = bigpool.tile([P, cfg.nchunks], f32, tag="sumP")
            sq_parts = bigpool.tile([P, cfg.nchunks], f32, tag="sqP")
            sq_scr = bigpool.tile([P, CHUNK], bf16, tag="sqscr")

            # ---------- input MLP ----------
            xv = v("x8", [F, NS], f8)
            for c in range(cfg.nchunks):
                cw = cfg.cw[c]
                xc8 = workpool.tile([F, CHUNK], f8, tag="xc8")
                nc.sync.dma_start(xc8[:, :cw], xv[:, c * CHUNK:c * CHUNK + cw])
                xc = workpool.tile([F, CHUNK], bf16, tag="xc")
                nc.vector.tensor_copy(xc[:, :cw], xc8[:, :cw])
                ps = psB.tile([P, CHUNK], f32, tag="psB")
                nc.tensor.matmul(out=ps[:, :cw], lhsT=w_in_t[:], rhs=xc[:, :cw],
                                 start=True, stop=True)
                nc.scalar.activation(hT[:, c * CHUNK:c * CHUNK + cw], ps[:, :cw],
                                     AF.Relu, bias=b_in_t[:, 0:1], scale=1.0)

            def emit_transpose_store(l):
                for b in range(nblk):
                    bw = min(P, NS - b * P)
                    pst = psT.tile([P, P], bf16, tag="psT")
                    nc.tensor.transpose(pst[:bw, :P], hT[:, b * P:b * P + bw],
                                        ident[:])
                    rm = workpool.tile([P, P], bf16, tag="rm")
                    nc.vector.tensor_copy(rm[:bw, :], pst[:bw, :P])
                    nc.sync.dma_start(h_shard[l][b * P:b * P + bw, :], rm[:bw, :])
                nc.gpsimd.collective_compute(
                    "AllGather", OP.bypass, replica_groups=[cores],
                    ins=[h_shard[l][:]], outs=[h_full[l][:]])
                if cfg.DEBUG:
                    nc.gpsimd.dma_start(dbg_h[l][:], h_full[l][:])

            emit_transpose_store(0)

            # ---------- RGCN layers ----------
            for l in range(L):
                root_i = L * R + l
                for c in range(cfg.nchunks):
                    cw = cfg.cw[c]
                    lo, hi, rlists = chunk_tiles[c]
                    nS = hi - lo
                    msg = msgpool.tile([P, nS, H], bf16, tag="msg")
                    for s in range(nS):
                        nc.gpsimd.indirect_dma_start(
                            out=msg[:, s, :], out_offset=None,
                            in_=h_full[l][:],
                            in_offset=bass.IndirectOffsetOnAxis(
                                ap=idx_t[:, lo + s:lo + s + 1], axis=0))

                    mean = meanpool.tile([P, R, CHUNK], bf16, tag="mean")
                    for r in range(R):
                        psa = psA.tile([P, CHUNK], f32, tag="psA")
                        for (tb, tn, w) in rlists[r]:
                            ww = min(WIN, cw - w * WIN)
                            for t in range(tn):
                                s = tb + t
                                hot = hotpool.tile([P, WIN], bf16, tag="hot")
                                nc.vector.tensor_scalar(
                                    out=hot[:, :ww], in0=iota_bf[:, :ww],
                                    scalar1=loc_t[:, s:s + 1],
                                    scalar2=alp_t[:, s:s + 1],
                                    op0=OP.is_equal, op1=OP.mult)
                                nc.tensor.matmul(
                                    out=psa[:, w * WIN:w * WIN + ww],
                                    lhsT=msg[:, s - lo, :], rhs=hot[:, :ww],
                                    start=(t == 0), stop=(t == tn - 1))
                        if r % 2 == 0:
                            nc.vector.tensor_copy(mean[:, r, :cw], psa[:, :cw])
                        else:
                            nc.scalar.copy(mean[:, r, :cw], psa[:, :cw])

                    psb = psB.tile([P, CHUNK], f32, tag="psB")
                    nc.tensor.matmul(out=psb[:, :cw], lhsT=relw_t[:, root_i, :],
                                     rhs=hT[:, c * CHUNK:c * CHUNK + cw],
                                     start=True, stop=False)
                    for r in range(R):
                        nc.tensor.matmul(out=psb[:, :cw],
                                         lhsT=relw_t[:, l * R + r, :],
                                         rhs=mean[:, r, :cw],
                                         start=False, stop=(r == R - 1))

                    nc.vector.tensor_scalar(
                        out=outb[:, c * CHUNK:c * CHUNK + cw], in0=psb[:, :cw],
                        scalar1=1.0, scalar2=None, op0=OP.mult, op1=OP.add,
                        accum_out=sum_parts[:, c:c + 1])
                    nc.scalar.activation(sq_scr[:, :cw], psb[:, :cw], AF.Square,
                                         accum_out=sq_parts[:, c:c + 1])

                # ---------- BatchNorm + ReLU ----------
                st = workpool.tile([H, 2], f32, tag="stats")
                nc.vector.reduce_sum(st[:, 0:1], sum_parts[:],
                                     axis=mybir.AxisListType.X)
                nc.vector.reduce_sum(st[:, 1:2], sq_parts[:],
                                     axis=mybir.AxisListType.X)
                nc.sync.dma_start(stats_in[:], st[:])
                nc.gpsimd.collective_compute(
                    "AllReduce", OP.add, replica_groups=[cores],
                    ins=[stats_in[:]], outs=[stats_out[:]])
                stg = workpool.tile([H, 8], f32, tag="stg")
                nc.sync.dma_start(stg[:, 0:2], stats_out[:])
                nc.vector.tensor_scalar(out=stg[:, 2:3], in0=stg[:, 0:1],
                                        scalar1=1.0 / N, scalar2=None,
                                        op0=OP.mult)
                nc.vector.tensor_scalar(out=stg[:, 3:4], in0=stg[:, 1:2],
                                        scalar1=1.0 / N, scalar2=None,
                                        op0=OP.mult)
                nc.vector.tensor_tensor(out=stg[:, 4:5], in0=stg[:, 2:3],
                                        in1=stg[:, 2:3], op=OP.mult)
                nc.vector.tensor_tensor(out=stg[:, 4:5], in0=stg[:, 3:4],
                                        in1=stg[:, 4:5], op=OP.subtract)
                nc.vector.tensor_scalar(out=stg[:, 4:5], in0=stg[:, 4:5],
                                        scalar1=cfg.EPS, scalar2=None,
                                        op0=OP.add)
                nc.scalar.sqrt(stg[:, 5:6], stg[:, 4:5])
                nc.vector.reciprocal(stg[:, 6:7], stg[:, 5:6])
                nc.vector.tensor_tensor(out=stg[:, 6:7], in0=stg[:, 6:7],
                                        in1=bng_t[:, l:l + 1], op=OP.mult)
                nc.vector.tensor_tensor(out=stg[:, 7:8], in0=stg[:, 6:7],
                                        in1=stg[:, 2:3], op=OP.mult)
                nc.vector.tensor_tensor(out=stg[:, 7:8], in0=bnb_t[:, l:l + 1],
                                        in1=stg[:, 7:8], op=OP.subtract)
                if cfg.DEBUG:
                    nc.sync.dma_start(dbg_outb[l], outb[:])
                    nc.sync.dma_start(dbg_stg[l], stg[:])
                nc.scalar.activation(hT[:], outb[:], AF.Relu,
                                     bias=stg[:, 7:8], scale=stg[:, 6:7])

                if l + 1 < L:
                    emit_transpose_store(l + 1)

            # ---------- global mean pool + output MLP ----------
            psp = psB.tile([G, CHUNK], f32, tag="psB")
            for b in range(nblk):
                bw = min(P, NS - b * P)
                pst = psT.tile([P, P], bf16, tag="psT")
                nc.tensor.transpose(pst[:bw, :P], hT[:, b * P:b * P + bw],
                                    ident[:])
                rm = workpool.tile([P, P], bf16, tag="rm")
                nc.vector.tensor_copy(rm[:bw, :], pst[:bw, :P])
                ind = hotpool.tile([P, G], bf16, tag="ind")
                nc.vector.tensor_scalar(out=ind[:bw, :], in0=iota_bf[:bw, :G],
                                        scalar1=gids_t[:bw, b:b + 1],
                                        scalar2=None, op0=OP.is_equal)
                nc.tensor.matmul(out=psp[:, :H], lhsT=ind[:bw, :],
                                 rhs=rm[:bw, :], start=(b == 0),
                                 stop=(b == nblk - 1))
            poolt = workpool.tile([G, H], f32, tag="poolt")
            nc.vector.tensor_copy(poolt[:], psp[:, :H])
            nc.sync.dma_start(pool_in[:], poolt[:])
            nc.gpsimd.collective_compute(
                "AllReduce", OP.add, replica_groups=[cores],
                ins=[pool_in[:]], outs=[pool_out[:]])
            poolg = workpool.tile([G, H], f32, tag="poolg")
            nc.sync.dma_start(poolg[:], pool_out[:])
            if cfg.DEBUG:
                nc.gpsimd.dma_start(dbg_pool[:], pool_out[:])

            pstT = psT.tile([P, G], f32, tag="psTf")
            nc.tensor.transpose(pstT[:, :G], poolg[:], identf[:])
            poolT = workpool.tile([P, G], f32, tag="poolT")
            nc.vector.tensor_copy(poolT[:], pstT[:, :G])

            psl = psB.tile([C, CHUNK], f32, tag="psB")
            nc.tensor.matmul(out=psl[:, :G], lhsT=w_out_t[:], rhs=poolT[:],
                             start=True, stop=True)
            logit = workpool.tile([C, G], f32, tag="logit")
            nc.vector.tensor_tensor(out=logit[:], in0=psl[:, :G], in1=invg_t[:],
                                    op=OP.mult)
            logit2 = workpool.tile([C, G], f32, tag="logit2")
            nc.scalar.activation(logit2[:], logit[:], AF.Sigmoid,
                                 bias=b_out_t[:, 0:1], scale=1.0)
            nc.sync.dma_start(out_d[:], logit2[:])

    # The bass_exec custom-call lowering re-serializes the (finalized,
    # immutable) BIR on every run_bass_kernel_spmd call (~0.19s for this
    # module). Memoize the serialization on this instance.
    _orig_tjb = nc.to_json_bytes
    _json_cache = []

    def _cached_tjb():
        if not _json_cache:
            _json_cache.append(_orig_tjb())
        return _json_cache[0]

    nc.to_json_bytes = _cached_tjb
    return nc


def _make_in_maps(cfg, plan, inputs):
    H, C, G, F, NS, R, L = cfg.H, cfg.C, cfg.G, cfg.F, cfg.NS, cfg.R, cfg.L
    S_total = plan["S_total"]
    OFF, TOT = _layout(cfg, S_total)
    x = np.asarray(inputs["x"], np.float32)
    batch = np.asarray(inputs["batch"])

    relw = np.empty((cfg.NMAT, P, H), BF16)
    rel_w = np.asarray(inputs["rel_w"], np.float32)
    root_w = np.asarray(inputs["root_w"], np.float32)
    for l in range(L):
        for r in range(R):
            relw[l * R + r] = rel_w[l, r].astype(BF16)
        relw[L * R + l] = root_w[l].astype(BF16)
    # [P, NMAT*H] partition-major, split into per-core partition slices
    relw_pT = np.ascontiguousarray(
        relw.transpose(1, 0, 2)).reshape(P, cfg.MATCOLS)

    bng = np.ascontiguousarray(np.asarray(inputs["bn_g"], np.float32).T)
    bnb = np.ascontiguousarray(np.asarray(inputs["bn_b"], np.float32).T)
    b_in = np.asarray(inputs["b_in"], np.float32).reshape(H, 1)
    b_out = np.asarray(inputs["b_out"], np.float32).reshape(C, 1)
    w_in = np.asarray(inputs["w_in"], np.float32).astype(BF16)
    w_out = np.asarray(inputs["w_out"], np.float32)
    invg = np.ascontiguousarray(np.broadcast_to(
        plan["inv_gcnt"].astype(np.float32)[None, :], (C, G)))

    def put(blob, name, arr):
        b = np.frombuffer(arr.tobytes(), np.uint8)
        blob[OFF[name]:OFF[name] + b.size] = b

    nblk = cfg.nblk
    in_maps = []
    for c in range(cfg.NC):
        lo, hi = c * NS, (c + 1) * NS
        xT8 = np.ascontiguousarray(x[lo:hi].T).astype(FP8)
        gids = np.full((P, nblk), 255, np.uint8)
        bseg = batch[lo:hi].astype(np.uint8)
        for b in range(nblk):
            bw = min(P, NS - b * P)
            gids[:bw, b] = bseg[b * P:b * P + bw]
        blob = np.zeros(TOT, np.uint8)
        Sh = (S_total + 1) // 2
        cnt_pad = np.zeros((P, 2 * Sh), np.uint8)
        cnt_pad[:, :S_total] = plan["cntA"][c]
        cnt_nib = cnt_pad[:, 0::2] | (cnt_pad[:, 1::2] << 4)
        put(blob, "x8", xT8)
        put(blob, "lo", np.ascontiguousarray(plan["loA"][c]))
        put(blob, "loc", np.ascontiguousarray(plan["locA"][c]))
        put(blob, "cnt", np.ascontiguousarray(cnt_nib))
        put(blob, "relw",
            np.ascontiguousarray(relw_pT[c * cfg.PSL:(c + 1) * cfg.PSL]))
        put(blob, "w_in", w_in)
        put(blob, "gids", gids)
        put(blob, "invg", invg)
        put(blob, "bng", bng)
        put(blob, "bnb", bnb)
        put(blob, "b_in", b_in)
        put(blob, "w_out", w_out)
        put(blob, "b_out", b_out)
        in_maps.append(dict(blob=blob))
    return in_maps


def _run(cfg, inputs, **kw):
    plan = _plan(cfg, np.asarray(inputs["edge_index"]),
                 np.asarray(inputs["edge_type"]), np.asarray(inputs["batch"]))
    nc = _build_nc(cfg, plan)
    if not nc.is_finalized():
        nc.finalize()
    in_maps = _make_in_maps(cfg, plan, inputs)
    res = run_bass_kernel_spmd(nc, in_maps, core_ids=list(range(cfg.NC)), **kw)
    out = np.asarray(res.results[0]["out"])
    if np.isnan(out).any():
        # Rare cold-start transient (observed once: NaN on a first call that
        # loaded a cached executable; repeats on the same program are clean).
        # One retry; a deterministic kernel bug would still NaN and fail.
        res = run_bass_kernel_spmd(nc, in_maps, core_ids=list(range(cfg.NC)),
                                   **kw)
        out = np.asarray(res.results[0]["out"])
    return np.ascontiguousarray(out.T.astype(np.float32)), res


def kernel(**inputs):
    cfg = Cfg()
    out, _ = _run(cfg, inputs)
    return out
